# revision 32
# baseline (speedup 1.0000x reference)
"""Trainium2 Bass kernel for windowed multi-head attention (nn_AttentionWindow).

Reference computation (B=64, N=197, DIM=768, H=12, HD=64):
    qkv  = x @ qkv_w.T + [q_bias, 0, v_bias]
    q, k, v = split(qkv);  q *= HD**-0.5
    attn = softmax(q @ k.T + rpb_table[rel_index])
    out  = (attn @ v) @ proj_w.T + proj_b

Sharding: data-parallel over batch across 8 NeuronCores (8 batches/core).

Per-core design (bf16 matmuls on TensorE, fp32 PSUM accumulation):
  - x pre-transposed on host to xT [768, 1576] bf16, loaded as 6 per-chunk
    SBUF tiles (per-chunk DMA deps let compute start ~6us earlier).
  - q,k feature-major into one resident [128, 12, 1576] tile, computed in
    4x394-token slices (no ragged tail matmuls).
  - v token-major per batch ([tokens, channels], 128+69 row chunks), bf16.
  - Attention per (head-pair, batch) item, software-pipelined (SKEW):
      * PE identity-matmul prefills the RAW rel-pos bias for the j2 token
        chunk into PSUM (one N=453 pass); j2 score matmuls accumulate onto
        it (start=False) so exp(S+B) comes straight out of the activation.
      * Scores S^T[j,i] row-packed (two heads on opposite 64-row halves).
      * ONE merged ScalarE exp over all 4 score quadrants -> P^T bf16.
      * The j1 exp(bias) multiply runs on the otherwise-idle GpSimd engine
        (SBUF-only op), keeping VectorE free.
      * O^T and softmax denominators: col-packed matmuls (h0 -> PSUM
        partitions 0:64, h1 -> 64:128 via tile_position), so reciprocal and
        normalize are ONE [128,197] VectorE op each.
  - Projection feature-major over 394-token slices; host transposes output.
"""
import sys
import functools

sys.path.insert(0, "/opt/trn_rl_repo")

import numpy as np
import ml_dtypes

import concourse.bass as bass  # noqa: E402
import concourse.bacc as bacc  # noqa: E402
import concourse.mybir as mybir  # noqa: E402
from concourse.tile import TileContext  # noqa: E402
from concourse.bass_utils import run_bass_kernel_spmd  # noqa: E402

F32 = mybir.dt.float32
BF16 = mybir.dt.bfloat16

NCORES = 8
B, NT, DIM = 64, 197, 768
H, HD = 12, 64
SCALE = HD ** -0.5  # 0.125, exact power of two -> folded into q weights
BPC = B // NCORES   # 8 batches per core
TOK = BPC * NT      # 1576 tokens per core
KC = DIM // 128     # 6
HP = H // 2         # 6 head pairs
NT2 = NT - 128      # 69 (second token chunk)
SKEW = 3            # attention software-pipeline depth (items)
SLW = 394           # token slice width (4 * 394 = 1576 exactly)
SLICES = [(s * SLW, (s + 1) * SLW) for s in range(4)]
MERGED_EXP = True   # one exp over all 4 score quadrants
EB_ENGINE = "gpsimd"  # engine for the j1 expB multiply: "vector"|"gpsimd"
PHASES = "all"
V_FM = False    # v feature-major + XBAR DMA transpose to token-major      # debug: "qkproj" | "qkv" | "all"


def build(qkv_bias_nonzero: bool, proj_bias_nonzero: bool):
    nc = bacc.Bacc("TRN2", target_bir_lowering=False, debug=False)

    xt = nc.dram_tensor("xt", [DIM, TOK], BF16, kind="ExternalInput")
    # host-blocked per out-chunk: [c][feature-part][kc*128+m], contiguous rows
    qkw = nc.dram_tensor("qkw", [2 * KC, 128, DIM], BF16, kind="ExternalInput")
    vw = nc.dram_tensor("vw", [DIM, DIM], BF16, kind="ExternalInput")
    pw = nc.dram_tensor("pw", [DIM, DIM], BF16, kind="ExternalInput")
    # expB per head: [h][j=0:128][jchunk][i]; jchunk 1 rows 69:128 = 1.0
    ebq = nc.dram_tensor("ebq", [H, 128, 2 * NT], BF16, kind="ExternalInput")
    # raw bias for even heads, bank-0 quadrant layout [hp][128][453]
    ebb = nc.dram_tensor("ebb", [HP, 128, 453], BF16, kind="ExternalInput")
    idm = nc.dram_tensor("idm", [128, 128], BF16, kind="ExternalInput")
    # slice-major blocks: [s][feature-part p][c*394+m]; host reassembles
    out = nc.dram_tensor("out", [4, 128, KC * SLW], BF16, kind="ExternalOutput")
    if qkv_bias_nonzero:
        qkb = nc.dram_tensor("qkb", [1, 2 * DIM], BF16, kind="ExternalInput")
        vb = nc.dram_tensor("vb", [1, DIM], BF16, kind="ExternalInput")
    if proj_bias_nonzero:
        pb = nc.dram_tensor("pb", [1, DIM], BF16, kind="ExternalInput")

    with TileContext(nc) as tc:
        with (
            tc.tile_pool(name="const", bufs=1) as constp,
            tc.tile_pool(name="vp", bufs=2 * BPC) as vp,
            tc.tile_pool(name="pp", bufs=SKEW + 2) as pp,
            tc.tile_pool(name="rcp", bufs=3) as rcp,
            tc.tile_pool(name="obp", bufs=1) as obp,
            tc.tile_pool(name="pb1", bufs=4, space="PSUM") as pb1,
            tc.tile_pool(name="sta", bufs=2, space="PSUM") as sta,
        ):
            # ---- resident inputs ----
            # DMA rings round-robin by issue index; split the inputs into
            # pieces issued in need-order so the first qk groups' data gets
            # the full aggregate bandwidth up front.
            xbt = [constp.tile([128, TOK], BF16, name=f"xb{kc}")
                   for kc in range(KC)]
            qkwc = [constp.tile([128, KC, 128], BF16, name=f"qkw{c}")
                    for c in range(2 * KC)]
            vwt = [constp.tile([128, DIM], BF16, name=f"vw{kc}")
                   for kc in range(KC)]
            pwt = [constp.tile([128, DIM], BF16, name=f"pw{kc}")
                   for kc in range(KC)]

            for kc in range(KC):
                nc.sync.dma_start(xbt[kc][:, :], xt[kc * 128:(kc + 1) * 128, :])
            for c in range(2 * KC):
                nc.sync.dma_start(
                    qkwc[c][:, :, :],
                    qkw[c, :, :].rearrange("p (k m) -> p k m", k=KC))
            for kc in range(KC):
                nc.sync.dma_start(vwt[kc][:, :], vw[kc * 128:(kc + 1) * 128, :])
            for kc in range(KC):
                nc.sync.dma_start(pwt[kc][:, :], pw[kc * 128:(kc + 1) * 128, :])
            ebq_s = {}
            for h in range(1, H, 2):
                ebq_s[h] = constp.tile([128, 2 * NT], BF16, name=f"ebq{h}")
                nc.sync.dma_start(ebq_s[h][:, :], ebq[h, :, :])
            ebb_s = [constp.tile([128, 453], BF16, name=f"ebb{hp}")
                     for hp in range(HP)]
            for hp in range(HP):
                nc.sync.dma_start(ebb_s[hp][:, :], ebb[hp, :, :])
            id_s = constp.tile([128, 128], BF16, name="id_s")
            nc.sync.dma_start(id_s[:, :], idm[:, :])
            ones_bf = constp.tile([128, 64], BF16, name="ones_bf")
            nc.gpsimd.memset(ones_bf[:, :], 1.0)
            if qkv_bias_nonzero:
                qkb_s = constp.tile([1, 2 * DIM], BF16, name="qkb_s")
                vb_s = constp.tile([1, DIM], BF16, name="vb_s")
                nc.sync.dma_start(qkb_s[:, :], qkb[:, :])
                nc.sync.dma_start(vb_s[:, :], vb[:, :])
            if proj_bias_nonzero:
                pb_s = constp.tile([1, DIM], BF16, name="pb_s")
                nc.sync.dma_start(pb_s[:, :], pb[:, :])
            if qkv_bias_nonzero or proj_bias_nonzero:
                ones_bfr = constp.tile([1, SLW], BF16, name="ones_bfr")
                nc.gpsimd.memset(ones_bfr[:, :], 1.0)
            # big resident activations: q,k and proj-rhs (bf16)
            qk_s = constp.tile([128, 2 * KC, TOK], BF16, name="qk_s")
            op_s = constp.tile([128, KC, TOK], BF16, name="op_s")
            if PHASES in ("qkproj", "qkv", "sa"):
                nc.gpsimd.memset(op_s[:, :, :], 0.0)

            # ---- q,k feature-major: 12 channel-chunks x 4 token slices ----
            for c in range(2 * KC):
                for t0, t1 in SLICES:
                    acc = pb1.tile([128, SLW], F32, name="acc_qk", tag="mm")
                    for kc in range(KC):
                        nc.tensor.matmul(
                            acc[:, :],
                            qkwc[c][:, kc, :],
                            xbt[kc][:, t0:t1],
                            start=(kc == 0),
                            stop=(kc == KC - 1) and not qkv_bias_nonzero,
                        )
                    if qkv_bias_nonzero:
                        nc.tensor.matmul(
                            acc[:, :],
                            qkb_s[0:1, c * 128:(c + 1) * 128],
                            ones_bfr[0:1, :],
                            start=False, stop=True,
                        )
                    nc.scalar.copy(qk_s[:, c, t0:t1], acc[:, :])

            # ---- v: [(128|69) tok, 768 ch] per batch ----
            vt = [[None, None] for _ in range(BPC)]
            if V_FM:
                # feature-major like qk (cheaper matmuls: no ragged 69-row
                # groups), then XBAR DMA-transpose to token-major tiles.
                vfm = constp.tile([128, KC, TOK + 88], BF16, name="vfm")
                nc.gpsimd.memset(vfm[:, :, TOK:], 0.0)
                for c in range(KC):
                    for t0, t1 in SLICES:
                        acc = pb1.tile([128, SLW], F32, name="acc_v", tag="mm")
                        for kc in range(KC):
                            nc.tensor.matmul(
                                acc[:, :],
                                vwt[kc][:, c * 128:(c + 1) * 128],
                                xbt[kc][:, t0:t1],
                                start=(kc == 0),
                                stop=(kc == KC - 1) and not qkv_bias_nonzero,
                            )
                        if qkv_bias_nonzero:
                            nc.tensor.matmul(
                                acc[:, :],
                                vb_s[0:1, c * 128:(c + 1) * 128],
                                ones_bfr[0:1, :],
                                start=False, stop=True,
                            )
                        nc.scalar.copy(vfm[:, c, t0:t1], acc[:, :])
                for b in range(BPC):
                    for tch in range(2):
                        toff = b * NT + tch * 128
                        t = vp.tile([128, DIM], BF16, name="v_t", tag="v")
                        for cc in range(KC):
                            nc.sync.dma_start(
                                t[:, cc * 128:(cc + 1) * 128],
                                vfm[:, cc, toff:toff + 128], transpose=True)
                        vt[b][tch] = t
            else:
                for b in range(BPC if PHASES != "qkproj" else 0):
                    for tch in range(2):
                        toff = b * NT + tch * 128
                        tlen = 128 if tch == 0 else NT2
                        t = vp.tile([128, DIM], BF16, name="v_t", tag="v")
                        for half in range(2):
                            n0, n1 = half * 384, (half + 1) * 384
                            acc = pb1.tile([128, 384], F32, name="acc_v",
                                           tag="mm")
                            for kc in range(KC):
                                nc.tensor.matmul(
                                    acc[0:tlen, :],
                                    xbt[kc][:, toff:toff + tlen],
                                    vwt[kc][:, n0:n1],
                                    start=(kc == 0),
                                    stop=(kc == KC - 1) and not qkv_bias_nonzero,
                                )
                            if qkv_bias_nonzero:
                                nc.tensor.matmul(
                                    acc[0:tlen, :],
                                    ones_bfr[0:1, 0:tlen],
                                    vb_s[0:1, n0:n1],
                                    start=False, stop=True,
                                )
                            nc.vector.tensor_copy(t[0:tlen, n0:n1],
                                                  acc[0:tlen, :])
                        vt[b][tch] = t

            # ---- attention, software-pipelined over (head-pair, batch) ----
            # st quadrants: [0:197]=j1h0 [256:453]=j1h1 [512:709]=j2h0
            # [768:965]=j2h1 (j2 quadrants pre-filled with raw bias).
            eb_eng = nc.gpsimd if EB_ENGINE == "gpsimd" else nc.vector

            def stage_a(b, hp, i):
                st = sta.tile([128, 1024], F32, name="st", tag="sta")
                q0 = qk_s[0:64, hp, b * NT:(b + 1) * NT]
                q1 = qk_s[64:128, hp, b * NT:(b + 1) * NT]
                # quadrants: [0:197]=j1h0 [256:453]=j2h0 (bank 0, row pos 0)
                #            [512:709]=j1h1 [768:965]=j2h1 (bank 1, row pos 64)
                # bank 0 is prefilled with h0's raw bias (identity matmul, row
                # pos 0 like the h0 scores); h0 scores accumulate onto it so
                # exp(S+B) comes straight out of the activation.
                nc.tensor.matmul(st[:, 0:453], id_s[:, :], ebb_s[hp][:, :],
                                 start=True, stop=False)
                nc.tensor.matmul(st[:, 0:NT],
                                 qk_s[0:64, KC + hp, b * NT:b * NT + 128],
                                 q0, start=False, stop=True)
                nc.tensor.matmul(st[:, 512:512 + NT],
                                 qk_s[64:128, KC + hp, b * NT:b * NT + 128],
                                 q1, start=True, stop=True)
                nc.tensor.matmul(st[0:NT2, 256:256 + NT],
                                 qk_s[0:64, KC + hp, b * NT + 128:(b + 1) * NT],
                                 q0, start=False, stop=True)
                nc.tensor.matmul(st[0:NT2, 768:768 + NT],
                                 qk_s[64:128, KC + hp, b * NT + 128:(b + 1) * NT],
                                 q1, start=True, stop=True)
                pj = pp.tile([128, 4, NT], BF16, name="pj", tag="p")
                if MERGED_EXP:
                    # one exp over all 4 quadrants (c = j1h0, j2h0, j1h1, j2h1)
                    nc.scalar.activation(
                        pj[:, :, :],
                        st[:, :].rearrange("p (c x) -> p c x", c=4)[:, :, 0:NT],
                        mybir.ActivationFunctionType.Exp)
                else:
                    nc.scalar.activation(
                        pj[:, 0:2, :],
                        st[:, 0:512].rearrange("p (c x) -> p c x", c=2)[:, :, 0:NT],
                        mybir.ActivationFunctionType.Exp)
                    nc.scalar.activation(
                        pj[:, 2:4, :],
                        st[:, 512:1024].rearrange(
                            "p (c x) -> p c x", c=2)[:, :, 0:NT],
                        mybir.ActivationFunctionType.Exp)
                # h1 expB multiply on gpsimd (h0 bias was added in PSUM)
                h1 = 2 * hp + 1
                eng0 = nc.gpsimd if EB_ENGINE == "gpsimd" else nc.vector
                pjf = pj[:, :, :].rearrange("p c x -> p (c x)")
                eng0.tensor_mul(pjf[:, 2 * NT:4 * NT], pjf[:, 2 * NT:4 * NT],
                                ebq_s[h1][:, :])
                return pj

            def stage_b(b, hp, pj):
                """O^T + denominators col-packed (h0->rows 0:64, h1->64:128),
                one reciprocal + one normalize on VectorE."""
                ot = pb1.tile([128, 512], F32, name="ot", tag="mm")
                h0s, h1s = 2 * hp * HD, (2 * hp + 1) * HD
                nc.tensor.matmul(ot[0:64, 0:NT],
                                 vt[b][0][:, h0s:h0s + HD],
                                 pj[:, 0, :], start=True, stop=False)
                nc.tensor.matmul(ot[64:128, 0:NT],
                                 vt[b][0][:, h1s:h1s + HD],
                                 pj[:, 2, :], start=True, stop=False,
                                 tile_position=(0, 64))
                nc.tensor.matmul(ot[0:64, 0:NT],
                                 vt[b][1][0:NT2, h0s:h0s + HD],
                                 pj[0:NT2, 1, :], start=False, stop=True)
                nc.tensor.matmul(ot[64:128, 0:NT],
                                 vt[b][1][0:NT2, h1s:h1s + HD],
                                 pj[0:NT2, 3, :], start=False, stop=True,
                                 tile_position=(0, 64))
                nc.tensor.matmul(ot[0:64, 256:256 + NT], ones_bf[:, :],
                                 pj[:, 0, :], start=True, stop=False)
                nc.tensor.matmul(ot[64:128, 256:256 + NT], ones_bf[:, :],
                                 pj[:, 2, :], start=True, stop=False,
                                 tile_position=(0, 64))
                nc.tensor.matmul(ot[0:64, 256:256 + NT], ones_bf[0:NT2, :],
                                 pj[0:NT2, 1, :], start=False, stop=True)
                nc.tensor.matmul(ot[64:128, 256:256 + NT], ones_bf[0:NT2, :],
                                 pj[0:NT2, 3, :], start=False, stop=True,
                                 tile_position=(0, 64))
                rc = rcp.tile([128, NT], F32, name="rc", tag="rc")
                nc.vector.reciprocal_approx_fast(
                    out=rc[:, :], in_=ot[:, 256:256 + NT])
                nc.vector.tensor_mul(
                    op_s[:, hp, b * NT:(b + 1) * NT],
                    ot[:, 0:NT], rc[:, :])

            # ---- projection group: one out-chunk c of token slice s ----
            obs = [obp.tile([128, KC, SLW], BF16, name=f"obt{s}")
                   for s in range(4)]

            def proj_group(c, s):
                t0, t1 = SLICES[s]
                acc = pb1.tile([128, SLW], F32, name="acc_p", tag="mm")
                for kp in range(KC):
                    nc.tensor.matmul(
                        acc[:, :],
                        pwt[kp][:, c * 128:(c + 1) * 128],
                        op_s[:, kp, t0:t1],
                        start=(kp == 0),
                        stop=(kp == KC - 1) and not proj_bias_nonzero,
                    )
                if proj_bias_nonzero:
                    nc.tensor.matmul(
                        acc[:, :],
                        pb_s[0:1, c * 128:(c + 1) * 128],
                        ones_bfr[0:1, :],
                        start=False, stop=True,
                    )
                nc.scalar.copy(obs[s][:, c, :], acc[:, :])
                if c == KC - 1:
                    # one wide DMA per slice: 4728B contiguous per partition
                    nc.sync.dma_start(
                        out[s, :, :],
                        obs[s][:, :, :].rearrange("p c m -> p (c m)"))

            items = ([(hp, b) for b in range(BPC) for hp in range(HP)]
                     if PHASES in ("all", "sa") else [])
            pend = {}

            def do_stage_b(j):
                stage_b(*pend.pop(j))
                jhp, jb = items[j]
                # token slice s = batches (2s, 2s+1): emit its projection as
                # soon as the last norm of batch 2s+1 is issued, so output
                # DMA streams during attention instead of trailing it.
                if jhp == HP - 1 and jb % 2 == 1:
                    for c in range(KC):
                        proj_group(c, jb // 2)

            for i, (hp, b) in enumerate(items):
                pend[i] = (b, hp, stage_a(b, hp, i))
                if i >= SKEW and PHASES != "sa":
                    do_stage_b(i - SKEW)
            if PHASES != "sa":
                for i in sorted(pend):
                    do_stage_b(i)
            if PHASES != "all":
                for s in range(4):
                    for c in range(KC):
                        proj_group(c, s)

    nc.compile()
    return nc


@functools.lru_cache(maxsize=4)
def _built(qkv_bias_nonzero: bool, proj_bias_nonzero: bool):
    return build(qkv_bias_nonzero, proj_bias_nonzero)


def prepare_inputs(x, qkv_w, q_bias, v_bias, rpb_table, proj_w, proj_b, rel_index):
    """Host-side prep: shard + transpose + fold scale + gather bias table."""
    x = np.asarray(x, dtype=np.float32)
    qkv_w = np.asarray(qkv_w, dtype=np.float32)
    q_bias = np.asarray(q_bias, dtype=np.float32)
    v_bias = np.asarray(v_bias, dtype=np.float32)
    rpb_table = np.asarray(rpb_table, dtype=np.float32)
    proj_w = np.asarray(proj_w, dtype=np.float32)
    proj_b = np.asarray(proj_b, dtype=np.float32)
    rel_index = np.asarray(rel_index)

    qw = qkv_w[0:DIM] * np.float32(SCALE)   # exact: SCALE is a power of two
    qkw_fm = np.concatenate([qw, qkv_w[DIM:2 * DIM]], axis=0).T  # [768, 1536]
    # block to [c][feature-part p][kc, m] so DMA rows are 1536B contiguous
    qkw_h = np.ascontiguousarray(
        qkw_fm.reshape(KC, 128, 2 * KC, 128).transpose(2, 1, 0, 3)
        .reshape(2 * KC, 128, DIM)).astype(ml_dtypes.bfloat16)
    vw_h = np.ascontiguousarray(qkv_w[2 * DIM:3 * DIM].T).astype(
        ml_dtypes.bfloat16)                                      # [768, 768]
    pw_h = np.ascontiguousarray(proj_w.T).astype(ml_dtypes.bfloat16)

    # bias[i, j, h] -> biasT[h, j, i]
    bias = rpb_table[rel_index].astype(np.float32)               # (197,197,12)
    biasT = bias.transpose(2, 1, 0)                              # (12, j, i)
    ebT = np.exp(biasT)
    # expB for j1 (j=0:128), both heads of each pair adjacent
    # per-head quadrant table: [h][j-part][jchunk][i]; j2 rows 69:128 -> 1.0
    ebq_h = np.ones((H, 128, 2, NT), dtype=np.float32)
    for h in range(H):
        ebq_h[h, :, 0, :] = ebT[h, 0:128, :]
        ebq_h[h, 0:NT2, 1, :] = ebT[h, 128:NT, :]
    ebq_h = ebq_h.reshape(H, 128, 2 * NT).astype(ml_dtypes.bfloat16)
    ebb_h = np.zeros((HP, 128, 453), dtype=np.float32)
    for hp in range(HP):
        ebb_h[hp, 0:128, 0:NT] = biasT[2 * hp, 0:128, :]
        ebb_h[hp, 0:NT2, 256:256 + NT] = biasT[2 * hp, 128:NT, :]
    ebb_h = ebb_h.astype(ml_dtypes.bfloat16)
    id_h = np.eye(128, dtype=ml_dtypes.bfloat16)

    qkv_bias_nonzero = bool(q_bias.any() or v_bias.any())
    proj_bias_nonzero = bool(proj_b.any())

    in_maps = []
    for i in range(NCORES):
        xs = x[i * BPC:(i + 1) * BPC].reshape(TOK, DIM)
        m = {
            "xt": np.ascontiguousarray(xs.T).astype(ml_dtypes.bfloat16),
            "qkw": qkw_h, "vw": vw_h, "pw": pw_h,
            "ebq": ebq_h, "ebb": ebb_h, "idm": id_h,
        }
        if qkv_bias_nonzero:
            m["qkb"] = np.ascontiguousarray(
                np.concatenate([q_bias * np.float32(SCALE),
                                np.zeros_like(q_bias)])[None, :],
                dtype=np.float32).astype(ml_dtypes.bfloat16)
            m["vb"] = np.ascontiguousarray(
                v_bias[None, :]).astype(ml_dtypes.bfloat16)
        if proj_bias_nonzero:
            m["pb"] = np.ascontiguousarray(
                proj_b[None, :], dtype=np.float32).astype(ml_dtypes.bfloat16)
        in_maps.append(m)
    return in_maps, qkv_bias_nonzero, proj_bias_nonzero


def kernel(x, qkv_w, q_bias, v_bias, rpb_table, proj_w, proj_b, rel_index):
    in_maps, qb_nz, pb_nz = prepare_inputs(
        x, qkv_w, q_bias, v_bias, rpb_table, proj_w, proj_b, rel_index)
    nc = _built(qb_nz, pb_nz)
    res = run_bass_kernel_spmd(nc, in_maps, core_ids=list(range(NCORES)))
    outs = []
    for i in range(NCORES):
        ob = res.results[i]["out"].astype(np.float32)     # [4, 128, 6*394]
        ofm = ob.reshape(4, 128, KC, SLW).transpose(2, 1, 0, 3).reshape(DIM, TOK)
        outs.append(ofm.T.reshape(BPC, NT, DIM))
    return np.concatenate(outs, axis=0).astype(np.float32)


# revision 33
# speedup vs baseline: 1.0016x; 1.0016x over previous
"""Trainium2 Bass kernel for windowed multi-head attention (nn_AttentionWindow).

Reference computation (B=64, N=197, DIM=768, H=12, HD=64):
    qkv  = x @ qkv_w.T + [q_bias, 0, v_bias]
    q, k, v = split(qkv);  q *= HD**-0.5
    attn = softmax(q @ k.T + rpb_table[rel_index])
    out  = (attn @ v) @ proj_w.T + proj_b

Sharding: data-parallel over batch across 8 NeuronCores (8 batches/core).

Per-core design (bf16 matmuls on TensorE, fp32 PSUM accumulation):
  - x pre-transposed on host to xT [768, 1576] bf16, loaded as 6 per-chunk
    SBUF tiles (per-chunk DMA deps let compute start ~6us earlier).
  - q,k feature-major into one resident [128, 12, 1576] tile, computed in
    4x394-token slices (no ragged tail matmuls).
  - v token-major per batch ([tokens, channels], 128+69 row chunks), bf16.
  - Attention per (head-pair, batch) item, software-pipelined (SKEW):
      * PE identity-matmul prefills the RAW rel-pos bias for the j2 token
        chunk into PSUM (one N=453 pass); j2 score matmuls accumulate onto
        it (start=False) so exp(S+B) comes straight out of the activation.
      * Scores S^T[j,i] row-packed (two heads on opposite 64-row halves).
      * ONE merged ScalarE exp over all 4 score quadrants -> P^T bf16.
      * The j1 exp(bias) multiply runs on the otherwise-idle GpSimd engine
        (SBUF-only op), keeping VectorE free.
      * O^T and softmax denominators: col-packed matmuls (h0 -> PSUM
        partitions 0:64, h1 -> 64:128 via tile_position), so reciprocal and
        normalize are ONE [128,197] VectorE op each.
  - Projection feature-major over 394-token slices; host transposes output.
"""
import sys
import functools

sys.path.insert(0, "/opt/trn_rl_repo")

import numpy as np
import ml_dtypes

import concourse.bass as bass  # noqa: E402
import concourse.bacc as bacc  # noqa: E402
import concourse.mybir as mybir  # noqa: E402
from concourse.tile import TileContext  # noqa: E402
from concourse.bass_utils import run_bass_kernel_spmd  # noqa: E402

F32 = mybir.dt.float32
BF16 = mybir.dt.bfloat16

NCORES = 8
B, NT, DIM = 64, 197, 768
H, HD = 12, 64
SCALE = HD ** -0.5  # 0.125, exact power of two -> folded into q weights
BPC = B // NCORES   # 8 batches per core
TOK = BPC * NT      # 1576 tokens per core
KC = DIM // 128     # 6
HP = H // 2         # 6 head pairs
NT2 = NT - 128      # 69 (second token chunk)
SKEW = 3            # attention software-pipeline depth (items)
SLW = 394           # token slice width (4 * 394 = 1576 exactly)
SLICES = [(s * SLW, (s + 1) * SLW) for s in range(4)]
MERGED_EXP = True   # one exp over all 4 score quadrants
EB_ENGINE = "gpsimd"  # engine for the j1 expB multiply: "vector"|"gpsimd"
PHASES = "all"
V_FM = False    # v feature-major + XBAR DMA transpose to token-major      # debug: "qkproj" | "qkv" | "all"


def build(qkv_bias_nonzero: bool, proj_bias_nonzero: bool):
    nc = bacc.Bacc("TRN2", target_bir_lowering=False, debug=False)

    xt = nc.dram_tensor("xt", [DIM, TOK], BF16, kind="ExternalInput")
    # host-blocked per out-chunk: [c][feature-part][kc*128+m], contiguous rows
    qkw = nc.dram_tensor("qkw", [2 * KC, 128, DIM], BF16, kind="ExternalInput")
    vw = nc.dram_tensor("vw", [DIM, DIM], BF16, kind="ExternalInput")
    pw = nc.dram_tensor("pw", [DIM, DIM], BF16, kind="ExternalInput")
    # expB per head: [h][j=0:128][jchunk][i]; jchunk 1 rows 69:128 = 1.0
    ebq = nc.dram_tensor("ebq", [H, 128, 2 * NT], BF16, kind="ExternalInput")
    # raw bias for even heads, bank-0 quadrant layout [hp][128][453]
    ebb = nc.dram_tensor("ebb", [HP, 128, 453], BF16, kind="ExternalInput")
    idm = nc.dram_tensor("idm", [128, 128], BF16, kind="ExternalInput")
    # slice-major blocks: [s][feature-part p][c*394+m]; host reassembles
    out = nc.dram_tensor("out", [4, 128, KC * SLW], BF16, kind="ExternalOutput")
    if qkv_bias_nonzero:
        qkb = nc.dram_tensor("qkb", [1, 2 * DIM], BF16, kind="ExternalInput")
        vb = nc.dram_tensor("vb", [1, DIM], BF16, kind="ExternalInput")
    if proj_bias_nonzero:
        pb = nc.dram_tensor("pb", [1, DIM], BF16, kind="ExternalInput")

    with TileContext(nc) as tc:
        with (
            tc.tile_pool(name="const", bufs=1) as constp,
            tc.tile_pool(name="vp", bufs=2 * BPC) as vp,
            tc.tile_pool(name="pp", bufs=SKEW + 2) as pp,
            tc.tile_pool(name="rcp", bufs=3) as rcp,
            tc.tile_pool(name="obp", bufs=1) as obp,
            tc.tile_pool(name="pb1", bufs=4, space="PSUM") as pb1,
            tc.tile_pool(name="sta", bufs=2, space="PSUM") as sta,
        ):
            # ---- resident inputs ----
            # DMA rings round-robin by issue index; split the inputs into
            # pieces issued in need-order so the first qk groups' data gets
            # the full aggregate bandwidth up front.
            xbt = [constp.tile([128, TOK], BF16, name=f"xb{kc}")
                   for kc in range(KC)]
            qkwc = [constp.tile([128, KC, 128], BF16, name=f"qkw{c}")
                    for c in range(2 * KC)]
            vwt = [constp.tile([128, DIM], BF16, name=f"vw{kc}")
                   for kc in range(KC)]
            pwt = [constp.tile([128, DIM], BF16, name=f"pw{kc}")
                   for kc in range(KC)]

            for kc in range(KC):
                nc.sync.dma_start(xbt[kc][:, :], xt[kc * 128:(kc + 1) * 128, :])
            for c in range(2 * KC):
                nc.sync.dma_start(
                    qkwc[c][:, :, :],
                    qkw[c, :, :].rearrange("p (k m) -> p k m", k=KC))
            for kc in range(KC):
                nc.sync.dma_start(vwt[kc][:, :], vw[kc * 128:(kc + 1) * 128, :])
            for kc in range(KC):
                nc.sync.dma_start(pwt[kc][:, :], pw[kc * 128:(kc + 1) * 128, :])
            ebq_s = {}
            for h in range(1, H, 2):
                ebq_s[h] = constp.tile([128, 2 * NT], BF16, name=f"ebq{h}")
                nc.sync.dma_start(ebq_s[h][:, :], ebq[h, :, :])
            ebb_s = [constp.tile([128, 453], BF16, name=f"ebb{hp}")
                     for hp in range(HP)]
            for hp in range(HP):
                nc.sync.dma_start(ebb_s[hp][:, :], ebb[hp, :, :])
            id_s = constp.tile([128, 128], BF16, name="id_s")
            nc.sync.dma_start(id_s[:, :], idm[:, :])
            ones_bf = constp.tile([128, 64], BF16, name="ones_bf")
            nc.gpsimd.memset(ones_bf[:, :], 1.0)
            if qkv_bias_nonzero:
                qkb_s = constp.tile([1, 2 * DIM], BF16, name="qkb_s")
                vb_s = constp.tile([1, DIM], BF16, name="vb_s")
                nc.sync.dma_start(qkb_s[:, :], qkb[:, :])
                nc.sync.dma_start(vb_s[:, :], vb[:, :])
            if proj_bias_nonzero:
                pb_s = constp.tile([1, DIM], BF16, name="pb_s")
                nc.sync.dma_start(pb_s[:, :], pb[:, :])
            if qkv_bias_nonzero or proj_bias_nonzero:
                ones_bfr = constp.tile([1, SLW], BF16, name="ones_bfr")
                nc.gpsimd.memset(ones_bfr[:, :], 1.0)
            # big resident activations: q,k and proj-rhs (bf16)
            qk_s = constp.tile([128, 2 * KC, TOK], BF16, name="qk_s")
            op_s = constp.tile([128, KC, TOK], BF16, name="op_s")
            if PHASES in ("qkproj", "qkv", "sa"):
                nc.gpsimd.memset(op_s[:, :, :], 0.0)

            # ---- q,k feature-major: 12 channel-chunks x 4 token slices ----
            for c in range(2 * KC):
                for t0, t1 in SLICES:
                    acc = pb1.tile([128, SLW], F32, name="acc_qk", tag="mm")
                    for kc in range(KC):
                        nc.tensor.matmul(
                            acc[:, :],
                            qkwc[c][:, kc, :],
                            xbt[kc][:, t0:t1],
                            start=(kc == 0),
                            stop=(kc == KC - 1) and not qkv_bias_nonzero,
                        )
                    if qkv_bias_nonzero:
                        nc.tensor.matmul(
                            acc[:, :],
                            qkb_s[0:1, c * 128:(c + 1) * 128],
                            ones_bfr[0:1, :],
                            start=False, stop=True,
                        )
                    nc.scalar.copy(qk_s[:, c, t0:t1], acc[:, :])

            # ---- v: [(128|69) tok, 768 ch] per batch ----
            vt = [[None, None] for _ in range(BPC)]
            if V_FM:
                # feature-major like qk (cheaper matmuls: no ragged 69-row
                # groups), then XBAR DMA-transpose to token-major tiles.
                vfm = constp.tile([128, KC, TOK + 88], BF16, name="vfm")
                nc.gpsimd.memset(vfm[:, :, TOK:], 0.0)
                for c in range(KC):
                    for t0, t1 in SLICES:
                        acc = pb1.tile([128, SLW], F32, name="acc_v", tag="mm")
                        for kc in range(KC):
                            nc.tensor.matmul(
                                acc[:, :],
                                vwt[kc][:, c * 128:(c + 1) * 128],
                                xbt[kc][:, t0:t1],
                                start=(kc == 0),
                                stop=(kc == KC - 1) and not qkv_bias_nonzero,
                            )
                        if qkv_bias_nonzero:
                            nc.tensor.matmul(
                                acc[:, :],
                                vb_s[0:1, c * 128:(c + 1) * 128],
                                ones_bfr[0:1, :],
                                start=False, stop=True,
                            )
                        nc.scalar.copy(vfm[:, c, t0:t1], acc[:, :])
                for b in range(BPC):
                    for tch in range(2):
                        toff = b * NT + tch * 128
                        t = vp.tile([128, DIM], BF16, name="v_t", tag="v")
                        for cc in range(KC):
                            nc.sync.dma_start(
                                t[:, cc * 128:(cc + 1) * 128],
                                vfm[:, cc, toff:toff + 128], transpose=True)
                        vt[b][tch] = t
            else:
                for b in range(BPC if PHASES != "qkproj" else 0):
                    for tch in range(2):
                        toff = b * NT + tch * 128
                        tlen = 128 if tch == 0 else NT2
                        t = vp.tile([128, DIM], BF16, name="v_t", tag="v")
                        for half in range(2):
                            n0, n1 = half * 384, (half + 1) * 384
                            acc = pb1.tile([128, 384], F32, name="acc_v",
                                           tag="mm")
                            for kc in range(KC):
                                nc.tensor.matmul(
                                    acc[0:tlen, :],
                                    xbt[kc][:, toff:toff + tlen],
                                    vwt[kc][:, n0:n1],
                                    start=(kc == 0),
                                    stop=(kc == KC - 1) and not qkv_bias_nonzero,
                                )
                            if qkv_bias_nonzero:
                                nc.tensor.matmul(
                                    acc[0:tlen, :],
                                    ones_bfr[0:1, 0:tlen],
                                    vb_s[0:1, n0:n1],
                                    start=False, stop=True,
                                )
                            nc.vector.tensor_copy(t[0:tlen, n0:n1],
                                                  acc[0:tlen, :])
                        vt[b][tch] = t

            # ---- attention, software-pipelined over (head-pair, batch) ----
            # st quadrants: [0:197]=j1h0 [256:453]=j1h1 [512:709]=j2h0
            # [768:965]=j2h1 (j2 quadrants pre-filled with raw bias).
            eb_eng = nc.gpsimd if EB_ENGINE == "gpsimd" else nc.vector

            def stage_a(b, hp, i):
                st = sta.tile([128, 1024], F32, name="st", tag="sta")
                q0 = qk_s[0:64, hp, b * NT:(b + 1) * NT]
                q1 = qk_s[64:128, hp, b * NT:(b + 1) * NT]
                # quadrants: [0:197]=j1h0 [256:453]=j2h0 (bank 0, row pos 0)
                #            [512:709]=j1h1 [768:965]=j2h1 (bank 1, row pos 64)
                # bank 0 is prefilled with h0's raw bias (identity matmul, row
                # pos 0 like the h0 scores); h0 scores accumulate onto it so
                # exp(S+B) comes straight out of the activation.
                nc.tensor.matmul(st[:, 0:453], id_s[:, :], ebb_s[hp][:, :],
                                 start=True, stop=False)
                nc.tensor.matmul(st[:, 0:NT],
                                 qk_s[0:64, KC + hp, b * NT:b * NT + 128],
                                 q0, start=False, stop=True)
                nc.tensor.matmul(st[:, 512:512 + NT],
                                 qk_s[64:128, KC + hp, b * NT:b * NT + 128],
                                 q1, start=True, stop=True)
                nc.tensor.matmul(st[0:NT2, 256:256 + NT],
                                 qk_s[0:64, KC + hp, b * NT + 128:(b + 1) * NT],
                                 q0, start=False, stop=True)
                nc.tensor.matmul(st[0:NT2, 768:768 + NT],
                                 qk_s[64:128, KC + hp, b * NT + 128:(b + 1) * NT],
                                 q1, start=True, stop=True)
                pj = pp.tile([128, 4, NT], BF16, name="pj", tag="p")
                if MERGED_EXP:
                    # one exp over all 4 quadrants (c = j1h0, j2h0, j1h1, j2h1)
                    nc.scalar.activation(
                        pj[:, :, :],
                        st[:, :].rearrange("p (c x) -> p c x", c=4)[:, :, 0:NT],
                        mybir.ActivationFunctionType.Exp)
                else:
                    nc.scalar.activation(
                        pj[:, 0:2, :],
                        st[:, 0:512].rearrange("p (c x) -> p c x", c=2)[:, :, 0:NT],
                        mybir.ActivationFunctionType.Exp)
                    nc.scalar.activation(
                        pj[:, 2:4, :],
                        st[:, 512:1024].rearrange(
                            "p (c x) -> p c x", c=2)[:, :, 0:NT],
                        mybir.ActivationFunctionType.Exp)
                # h1 expB multiply on gpsimd (h0 bias was added in PSUM)
                h1 = 2 * hp + 1
                eng0 = nc.gpsimd if EB_ENGINE == "gpsimd" else nc.vector
                pjf = pj[:, :, :].rearrange("p c x -> p (c x)")
                eng0.tensor_mul(pjf[:, 2 * NT:4 * NT], pjf[:, 2 * NT:4 * NT],
                                ebq_s[h1][:, :])
                return pj

            def stage_b(b, hp, pj):
                """O^T + denominators col-packed (h0->rows 0:64, h1->64:128),
                one reciprocal + one normalize on VectorE."""
                ot = pb1.tile([128, 512], F32, name="ot", tag="mm")
                h0s, h1s = 2 * hp * HD, (2 * hp + 1) * HD
                nc.tensor.matmul(ot[0:64, 0:NT],
                                 vt[b][0][:, h0s:h0s + HD],
                                 pj[:, 0, :], start=True, stop=False)
                nc.tensor.matmul(ot[64:128, 0:NT],
                                 vt[b][0][:, h1s:h1s + HD],
                                 pj[:, 2, :], start=True, stop=False,
                                 tile_position=(0, 64))
                nc.tensor.matmul(ot[0:64, 0:NT],
                                 vt[b][1][0:NT2, h0s:h0s + HD],
                                 pj[0:NT2, 1, :], start=False, stop=True)
                nc.tensor.matmul(ot[64:128, 0:NT],
                                 vt[b][1][0:NT2, h1s:h1s + HD],
                                 pj[0:NT2, 3, :], start=False, stop=True,
                                 tile_position=(0, 64))
                nc.tensor.matmul(ot[0:64, 256:256 + NT], ones_bf[:, :],
                                 pj[:, 0, :], start=True, stop=False)
                nc.tensor.matmul(ot[64:128, 256:256 + NT], ones_bf[:, :],
                                 pj[:, 2, :], start=True, stop=False,
                                 tile_position=(0, 64))
                nc.tensor.matmul(ot[0:64, 256:256 + NT], ones_bf[0:NT2, :],
                                 pj[0:NT2, 1, :], start=False, stop=True)
                nc.tensor.matmul(ot[64:128, 256:256 + NT], ones_bf[0:NT2, :],
                                 pj[0:NT2, 3, :], start=False, stop=True,
                                 tile_position=(0, 64))
                rc = rcp.tile([128, NT], F32, name="rc", tag="rc")
                nc.vector.reciprocal_approx_fast(
                    out=rc[:, :], in_=ot[:, 256:256 + NT])
                nc.vector.tensor_mul(
                    op_s[:, hp, b * NT:(b + 1) * NT],
                    ot[:, 0:NT], rc[:, :])

            # ---- projection group: one out-chunk c of token slice s ----
            obs = [obp.tile([128, KC, SLW], BF16, name=f"obt{s}")
                   for s in range(4)]

            def proj_group(c, s):
                t0, t1 = SLICES[s]
                acc = pb1.tile([128, SLW], F32, name="acc_p", tag="mm")
                for kp in range(KC):
                    nc.tensor.matmul(
                        acc[:, :],
                        pwt[kp][:, c * 128:(c + 1) * 128],
                        op_s[:, kp, t0:t1],
                        start=(kp == 0),
                        stop=(kp == KC - 1) and not proj_bias_nonzero,
                    )
                if proj_bias_nonzero:
                    nc.tensor.matmul(
                        acc[:, :],
                        pb_s[0:1, c * 128:(c + 1) * 128],
                        ones_bfr[0:1, :],
                        start=False, stop=True,
                    )
                nc.scalar.copy(obs[s][:, c, :], acc[:, :])
                if c in (2, KC - 1):
                    c0 = 0 if c == 2 else 3
                    nc.sync.dma_start(
                        out[s, :, c0 * SLW:(c0 + 3) * SLW],
                        obs[s][:, c0:c0 + 3, :].rearrange("p c m -> p (c m)"))

            items = ([(hp, b) for b in range(BPC) for hp in range(HP)]
                     if PHASES in ("all", "sa") else [])
            pend = {}

            def do_stage_b(j):
                stage_b(*pend.pop(j))
                jhp, jb = items[j]
                # token slice s = batches (2s, 2s+1): emit its projection as
                # soon as the last norm of batch 2s+1 is issued, so output
                # DMA streams during attention instead of trailing it.
                if jhp == HP - 1 and jb % 2 == 1:
                    for c in range(KC):
                        proj_group(c, jb // 2)

            for i, (hp, b) in enumerate(items):
                pend[i] = (b, hp, stage_a(b, hp, i))
                if i >= SKEW and PHASES != "sa":
                    do_stage_b(i - SKEW)
            if PHASES != "sa":
                for i in sorted(pend):
                    do_stage_b(i)
            if PHASES != "all":
                for s in range(4):
                    for c in range(KC):
                        proj_group(c, s)

    nc.compile()
    return nc


@functools.lru_cache(maxsize=4)
def _built(qkv_bias_nonzero: bool, proj_bias_nonzero: bool):
    return build(qkv_bias_nonzero, proj_bias_nonzero)


def prepare_inputs(x, qkv_w, q_bias, v_bias, rpb_table, proj_w, proj_b, rel_index):
    """Host-side prep: shard + transpose + fold scale + gather bias table."""
    x = np.asarray(x, dtype=np.float32)
    qkv_w = np.asarray(qkv_w, dtype=np.float32)
    q_bias = np.asarray(q_bias, dtype=np.float32)
    v_bias = np.asarray(v_bias, dtype=np.float32)
    rpb_table = np.asarray(rpb_table, dtype=np.float32)
    proj_w = np.asarray(proj_w, dtype=np.float32)
    proj_b = np.asarray(proj_b, dtype=np.float32)
    rel_index = np.asarray(rel_index)

    qw = qkv_w[0:DIM] * np.float32(SCALE)   # exact: SCALE is a power of two
    qkw_fm = np.concatenate([qw, qkv_w[DIM:2 * DIM]], axis=0).T  # [768, 1536]
    # block to [c][feature-part p][kc, m] so DMA rows are 1536B contiguous
    qkw_h = np.ascontiguousarray(
        qkw_fm.reshape(KC, 128, 2 * KC, 128).transpose(2, 1, 0, 3)
        .reshape(2 * KC, 128, DIM)).astype(ml_dtypes.bfloat16)
    vw_h = np.ascontiguousarray(qkv_w[2 * DIM:3 * DIM].T).astype(
        ml_dtypes.bfloat16)                                      # [768, 768]
    pw_h = np.ascontiguousarray(proj_w.T).astype(ml_dtypes.bfloat16)

    # bias[i, j, h] -> biasT[h, j, i]
    bias = rpb_table[rel_index].astype(np.float32)               # (197,197,12)
    biasT = bias.transpose(2, 1, 0)                              # (12, j, i)
    ebT = np.exp(biasT)
    # expB for j1 (j=0:128), both heads of each pair adjacent
    # per-head quadrant table: [h][j-part][jchunk][i]; j2 rows 69:128 -> 1.0
    ebq_h = np.ones((H, 128, 2, NT), dtype=np.float32)
    for h in range(H):
        ebq_h[h, :, 0, :] = ebT[h, 0:128, :]
        ebq_h[h, 0:NT2, 1, :] = ebT[h, 128:NT, :]
    ebq_h = ebq_h.reshape(H, 128, 2 * NT).astype(ml_dtypes.bfloat16)
    ebb_h = np.zeros((HP, 128, 453), dtype=np.float32)
    for hp in range(HP):
        ebb_h[hp, 0:128, 0:NT] = biasT[2 * hp, 0:128, :]
        ebb_h[hp, 0:NT2, 256:256 + NT] = biasT[2 * hp, 128:NT, :]
    ebb_h = ebb_h.astype(ml_dtypes.bfloat16)
    id_h = np.eye(128, dtype=ml_dtypes.bfloat16)

    qkv_bias_nonzero = bool(q_bias.any() or v_bias.any())
    proj_bias_nonzero = bool(proj_b.any())

    in_maps = []
    for i in range(NCORES):
        xs = x[i * BPC:(i + 1) * BPC].reshape(TOK, DIM)
        m = {
            "xt": np.ascontiguousarray(xs.T).astype(ml_dtypes.bfloat16),
            "qkw": qkw_h, "vw": vw_h, "pw": pw_h,
            "ebq": ebq_h, "ebb": ebb_h, "idm": id_h,
        }
        if qkv_bias_nonzero:
            m["qkb"] = np.ascontiguousarray(
                np.concatenate([q_bias * np.float32(SCALE),
                                np.zeros_like(q_bias)])[None, :],
                dtype=np.float32).astype(ml_dtypes.bfloat16)
            m["vb"] = np.ascontiguousarray(
                v_bias[None, :]).astype(ml_dtypes.bfloat16)
        if proj_bias_nonzero:
            m["pb"] = np.ascontiguousarray(
                proj_b[None, :], dtype=np.float32).astype(ml_dtypes.bfloat16)
        in_maps.append(m)
    return in_maps, qkv_bias_nonzero, proj_bias_nonzero


def kernel(x, qkv_w, q_bias, v_bias, rpb_table, proj_w, proj_b, rel_index):
    in_maps, qb_nz, pb_nz = prepare_inputs(
        x, qkv_w, q_bias, v_bias, rpb_table, proj_w, proj_b, rel_index)
    nc = _built(qb_nz, pb_nz)
    res = run_bass_kernel_spmd(nc, in_maps, core_ids=list(range(NCORES)))
    outs = []
    for i in range(NCORES):
        ob = res.results[i]["out"].astype(np.float32)     # [4, 128, 6*394]
        ofm = ob.reshape(4, 128, KC, SLW).transpose(2, 1, 0, 3).reshape(DIM, TOK)
        outs.append(ofm.T.reshape(BPC, NT, DIM))
    return np.concatenate(outs, axis=0).astype(np.float32)


# revision 34
# speedup vs baseline: 1.0027x; 1.0011x over previous
"""Trainium2 Bass kernel for windowed multi-head attention (nn_AttentionWindow).

Reference computation (B=64, N=197, DIM=768, H=12, HD=64):
    qkv  = x @ qkv_w.T + [q_bias, 0, v_bias]
    q, k, v = split(qkv);  q *= HD**-0.5
    attn = softmax(q @ k.T + rpb_table[rel_index])
    out  = (attn @ v) @ proj_w.T + proj_b

Sharding: data-parallel over batch across 8 NeuronCores (8 batches/core).

Per-core design (bf16 matmuls on TensorE, fp32 PSUM accumulation):
  - x pre-transposed on host to xT [768, 1576] bf16, loaded as 6 per-chunk
    SBUF tiles (per-chunk DMA deps let compute start ~6us earlier).
  - q,k feature-major into one resident [128, 12, 1576] tile, computed in
    4x394-token slices (no ragged tail matmuls).
  - v token-major per batch ([tokens, channels], 128+69 row chunks), bf16.
  - Attention per (head-pair, batch) item, software-pipelined (SKEW):
      * PE identity-matmul prefills the RAW rel-pos bias for the j2 token
        chunk into PSUM (one N=453 pass); j2 score matmuls accumulate onto
        it (start=False) so exp(S+B) comes straight out of the activation.
      * Scores S^T[j,i] row-packed (two heads on opposite 64-row halves).
      * ONE merged ScalarE exp over all 4 score quadrants -> P^T bf16.
      * The j1 exp(bias) multiply runs on the otherwise-idle GpSimd engine
        (SBUF-only op), keeping VectorE free.
      * O^T and softmax denominators: col-packed matmuls (h0 -> PSUM
        partitions 0:64, h1 -> 64:128 via tile_position), so reciprocal and
        normalize are ONE [128,197] VectorE op each.
  - Projection feature-major over 394-token slices; host transposes output.
"""
import sys
import functools

sys.path.insert(0, "/opt/trn_rl_repo")

import numpy as np
import ml_dtypes

import concourse.bass as bass  # noqa: E402
import concourse.bacc as bacc  # noqa: E402
import concourse.mybir as mybir  # noqa: E402
from concourse.tile import TileContext  # noqa: E402
from concourse.bass_utils import run_bass_kernel_spmd  # noqa: E402

F32 = mybir.dt.float32
BF16 = mybir.dt.bfloat16

NCORES = 8
B, NT, DIM = 64, 197, 768
H, HD = 12, 64
SCALE = HD ** -0.5  # 0.125, exact power of two -> folded into q weights
BPC = B // NCORES   # 8 batches per core
TOK = BPC * NT      # 1576 tokens per core
KC = DIM // 128     # 6
HP = H // 2         # 6 head pairs
NT2 = NT - 128      # 69 (second token chunk)
SKEW = 3            # attention software-pipeline depth (items)
SLW = 394           # token slice width (4 * 394 = 1576 exactly)
SLICES = [(s * SLW, (s + 1) * SLW) for s in range(4)]
MERGED_EXP = True   # one exp over all 4 score quadrants
EB_ENGINE = "gpsimd"  # engine for the j1 expB multiply: "vector"|"gpsimd"
PHASES = "all"
V_FM = False    # v feature-major + XBAR DMA transpose to token-major      # debug: "qkproj" | "qkv" | "all"


def build(qkv_bias_nonzero: bool, proj_bias_nonzero: bool):
    nc = bacc.Bacc("TRN2", target_bir_lowering=False, debug=False)

    xt = nc.dram_tensor("xt", [DIM, TOK], BF16, kind="ExternalInput")
    # host-blocked per out-chunk: [c][feature-part][kc*128+m], contiguous rows
    qkw = nc.dram_tensor("qkw", [2 * KC, 128, DIM], BF16, kind="ExternalInput")
    vw = nc.dram_tensor("vw", [DIM, DIM], BF16, kind="ExternalInput")
    pw = nc.dram_tensor("pw", [DIM, DIM], BF16, kind="ExternalInput")
    # expB per head: [h][j=0:128][jchunk][i]; jchunk 1 rows 69:128 = 1.0
    ebq = nc.dram_tensor("ebq", [H, 128, 2 * NT], BF16, kind="ExternalInput")
    # raw bias for even heads, bank-0 quadrant layout [hp][128][453]
    ebb = nc.dram_tensor("ebb", [HP, 128, 453], BF16, kind="ExternalInput")
    idm = nc.dram_tensor("idm", [128, 128], BF16, kind="ExternalInput")
    out = nc.dram_tensor("out", [DIM, TOK], BF16, kind="ExternalOutput")
    if qkv_bias_nonzero:
        qkb = nc.dram_tensor("qkb", [1, 2 * DIM], BF16, kind="ExternalInput")
        vb = nc.dram_tensor("vb", [1, DIM], BF16, kind="ExternalInput")
    if proj_bias_nonzero:
        pb = nc.dram_tensor("pb", [1, DIM], BF16, kind="ExternalInput")

    with TileContext(nc) as tc:
        with (
            tc.tile_pool(name="const", bufs=1) as constp,
            tc.tile_pool(name="vp", bufs=2 * BPC) as vp,
            tc.tile_pool(name="pp", bufs=SKEW + 2) as pp,
            tc.tile_pool(name="rcp", bufs=3) as rcp,
            tc.tile_pool(name="obp", bufs=4) as obp,
            tc.tile_pool(name="pb1", bufs=4, space="PSUM") as pb1,
            tc.tile_pool(name="sta", bufs=2, space="PSUM") as sta,
        ):
            # ---- resident inputs ----
            # DMA rings round-robin by issue index; split the inputs into
            # pieces issued in need-order so the first qk groups' data gets
            # the full aggregate bandwidth up front.
            xbt = [constp.tile([128, TOK], BF16, name=f"xb{kc}")
                   for kc in range(KC)]
            qkwc = [constp.tile([128, KC, 128], BF16, name=f"qkw{c}")
                    for c in range(2 * KC)]
            vwt = [constp.tile([128, DIM], BF16, name=f"vw{kc}")
                   for kc in range(KC)]
            pwt = [constp.tile([128, DIM], BF16, name=f"pw{kc}")
                   for kc in range(KC)]

            for kc in range(KC):
                nc.sync.dma_start(xbt[kc][:, :], xt[kc * 128:(kc + 1) * 128, :])
            for c in range(2 * KC):
                nc.sync.dma_start(
                    qkwc[c][:, :, :],
                    qkw[c, :, :].rearrange("p (k m) -> p k m", k=KC))
            for kc in range(KC):
                nc.sync.dma_start(vwt[kc][:, :], vw[kc * 128:(kc + 1) * 128, :])
            for kc in range(KC):
                nc.sync.dma_start(pwt[kc][:, :], pw[kc * 128:(kc + 1) * 128, :])
            ebq_s = {}
            for h in range(1, H, 2):
                ebq_s[h] = constp.tile([128, 2 * NT], BF16, name=f"ebq{h}")
                nc.sync.dma_start(ebq_s[h][:, :], ebq[h, :, :])
            ebb_s = [constp.tile([128, 453], BF16, name=f"ebb{hp}")
                     for hp in range(HP)]
            for hp in range(HP):
                nc.sync.dma_start(ebb_s[hp][:, :], ebb[hp, :, :])
            id_s = constp.tile([128, 128], BF16, name="id_s")
            nc.sync.dma_start(id_s[:, :], idm[:, :])
            ones_bf = constp.tile([128, 64], BF16, name="ones_bf")
            nc.gpsimd.memset(ones_bf[:, :], 1.0)
            if qkv_bias_nonzero:
                qkb_s = constp.tile([1, 2 * DIM], BF16, name="qkb_s")
                vb_s = constp.tile([1, DIM], BF16, name="vb_s")
                nc.sync.dma_start(qkb_s[:, :], qkb[:, :])
                nc.sync.dma_start(vb_s[:, :], vb[:, :])
            if proj_bias_nonzero:
                pb_s = constp.tile([1, DIM], BF16, name="pb_s")
                nc.sync.dma_start(pb_s[:, :], pb[:, :])
            if qkv_bias_nonzero or proj_bias_nonzero:
                ones_bfr = constp.tile([1, SLW], BF16, name="ones_bfr")
                nc.gpsimd.memset(ones_bfr[:, :], 1.0)
            # big resident activations: q,k and proj-rhs (bf16)
            qk_s = constp.tile([128, 2 * KC, TOK], BF16, name="qk_s")
            op_s = constp.tile([128, KC, TOK], BF16, name="op_s")
            if PHASES in ("qkproj", "qkv", "sa"):
                nc.gpsimd.memset(op_s[:, :, :], 0.0)

            # ---- q,k feature-major: 12 channel-chunks x 4 token slices ----
            for c in range(2 * KC):
                for t0, t1 in SLICES:
                    acc = pb1.tile([128, SLW], F32, name="acc_qk", tag="mm")
                    for kc in range(KC):
                        nc.tensor.matmul(
                            acc[:, :],
                            qkwc[c][:, kc, :],
                            xbt[kc][:, t0:t1],
                            start=(kc == 0),
                            stop=(kc == KC - 1) and not qkv_bias_nonzero,
                        )
                    if qkv_bias_nonzero:
                        nc.tensor.matmul(
                            acc[:, :],
                            qkb_s[0:1, c * 128:(c + 1) * 128],
                            ones_bfr[0:1, :],
                            start=False, stop=True,
                        )
                    nc.scalar.copy(qk_s[:, c, t0:t1], acc[:, :])

            # ---- v: [(128|69) tok, 768 ch] per batch ----
            vt = [[None, None] for _ in range(BPC)]
            if V_FM:
                # feature-major like qk (cheaper matmuls: no ragged 69-row
                # groups), then XBAR DMA-transpose to token-major tiles.
                vfm = constp.tile([128, KC, TOK + 88], BF16, name="vfm")
                nc.gpsimd.memset(vfm[:, :, TOK:], 0.0)
                for c in range(KC):
                    for t0, t1 in SLICES:
                        acc = pb1.tile([128, SLW], F32, name="acc_v", tag="mm")
                        for kc in range(KC):
                            nc.tensor.matmul(
                                acc[:, :],
                                vwt[kc][:, c * 128:(c + 1) * 128],
                                xbt[kc][:, t0:t1],
                                start=(kc == 0),
                                stop=(kc == KC - 1) and not qkv_bias_nonzero,
                            )
                        if qkv_bias_nonzero:
                            nc.tensor.matmul(
                                acc[:, :],
                                vb_s[0:1, c * 128:(c + 1) * 128],
                                ones_bfr[0:1, :],
                                start=False, stop=True,
                            )
                        nc.scalar.copy(vfm[:, c, t0:t1], acc[:, :])
                for b in range(BPC):
                    for tch in range(2):
                        toff = b * NT + tch * 128
                        t = vp.tile([128, DIM], BF16, name="v_t", tag="v")
                        for cc in range(KC):
                            nc.sync.dma_start(
                                t[:, cc * 128:(cc + 1) * 128],
                                vfm[:, cc, toff:toff + 128], transpose=True)
                        vt[b][tch] = t
            else:
                for b in range(BPC if PHASES != "qkproj" else 0):
                    for tch in range(2):
                        toff = b * NT + tch * 128
                        tlen = 128 if tch == 0 else NT2
                        t = vp.tile([128, DIM], BF16, name="v_t", tag="v")
                        for half in range(2):
                            n0, n1 = half * 384, (half + 1) * 384
                            acc = pb1.tile([128, 384], F32, name="acc_v",
                                           tag="mm")
                            for kc in range(KC):
                                nc.tensor.matmul(
                                    acc[0:tlen, :],
                                    xbt[kc][:, toff:toff + tlen],
                                    vwt[kc][:, n0:n1],
                                    start=(kc == 0),
                                    stop=(kc == KC - 1) and not qkv_bias_nonzero,
                                )
                            if qkv_bias_nonzero:
                                nc.tensor.matmul(
                                    acc[0:tlen, :],
                                    ones_bfr[0:1, 0:tlen],
                                    vb_s[0:1, n0:n1],
                                    start=False, stop=True,
                                )
                            nc.vector.tensor_copy(t[0:tlen, n0:n1],
                                                  acc[0:tlen, :])
                        vt[b][tch] = t

            # ---- attention, software-pipelined over (head-pair, batch) ----
            # st quadrants: [0:197]=j1h0 [256:453]=j1h1 [512:709]=j2h0
            # [768:965]=j2h1 (j2 quadrants pre-filled with raw bias).
            eb_eng = nc.gpsimd if EB_ENGINE == "gpsimd" else nc.vector

            def stage_a(b, hp, i):
                st = sta.tile([128, 1024], F32, name="st", tag="sta")
                q0 = qk_s[0:64, hp, b * NT:(b + 1) * NT]
                q1 = qk_s[64:128, hp, b * NT:(b + 1) * NT]
                # quadrants: [0:197]=j1h0 [256:453]=j2h0 (bank 0, row pos 0)
                #            [512:709]=j1h1 [768:965]=j2h1 (bank 1, row pos 64)
                # bank 0 is prefilled with h0's raw bias (identity matmul, row
                # pos 0 like the h0 scores); h0 scores accumulate onto it so
                # exp(S+B) comes straight out of the activation.
                nc.tensor.matmul(st[:, 0:453], id_s[:, :], ebb_s[hp][:, :],
                                 start=True, stop=False)
                nc.tensor.matmul(st[:, 0:NT],
                                 qk_s[0:64, KC + hp, b * NT:b * NT + 128],
                                 q0, start=False, stop=True)
                nc.tensor.matmul(st[:, 512:512 + NT],
                                 qk_s[64:128, KC + hp, b * NT:b * NT + 128],
                                 q1, start=True, stop=True)
                nc.tensor.matmul(st[0:NT2, 256:256 + NT],
                                 qk_s[0:64, KC + hp, b * NT + 128:(b + 1) * NT],
                                 q0, start=False, stop=True)
                nc.tensor.matmul(st[0:NT2, 768:768 + NT],
                                 qk_s[64:128, KC + hp, b * NT + 128:(b + 1) * NT],
                                 q1, start=True, stop=True)
                pj = pp.tile([128, 4, NT], BF16, name="pj", tag="p")
                if MERGED_EXP:
                    # one exp over all 4 quadrants (c = j1h0, j2h0, j1h1, j2h1)
                    nc.scalar.activation(
                        pj[:, :, :],
                        st[:, :].rearrange("p (c x) -> p c x", c=4)[:, :, 0:NT],
                        mybir.ActivationFunctionType.Exp)
                else:
                    nc.scalar.activation(
                        pj[:, 0:2, :],
                        st[:, 0:512].rearrange("p (c x) -> p c x", c=2)[:, :, 0:NT],
                        mybir.ActivationFunctionType.Exp)
                    nc.scalar.activation(
                        pj[:, 2:4, :],
                        st[:, 512:1024].rearrange(
                            "p (c x) -> p c x", c=2)[:, :, 0:NT],
                        mybir.ActivationFunctionType.Exp)
                # h1 expB multiply on gpsimd (h0 bias was added in PSUM)
                h1 = 2 * hp + 1
                eng0 = nc.gpsimd if EB_ENGINE == "gpsimd" else nc.vector
                pjf = pj[:, :, :].rearrange("p c x -> p (c x)")
                eng0.tensor_mul(pjf[:, 2 * NT:4 * NT], pjf[:, 2 * NT:4 * NT],
                                ebq_s[h1][:, :])
                return pj

            def stage_b(b, hp, pj):
                """O^T + denominators col-packed (h0->rows 0:64, h1->64:128),
                one reciprocal + one normalize on VectorE."""
                ot = pb1.tile([128, 512], F32, name="ot", tag="mm")
                h0s, h1s = 2 * hp * HD, (2 * hp + 1) * HD
                nc.tensor.matmul(ot[0:64, 0:NT],
                                 vt[b][0][:, h0s:h0s + HD],
                                 pj[:, 0, :], start=True, stop=False)
                nc.tensor.matmul(ot[64:128, 0:NT],
                                 vt[b][0][:, h1s:h1s + HD],
                                 pj[:, 2, :], start=True, stop=False,
                                 tile_position=(0, 64))
                nc.tensor.matmul(ot[0:64, 0:NT],
                                 vt[b][1][0:NT2, h0s:h0s + HD],
                                 pj[0:NT2, 1, :], start=False, stop=True)
                nc.tensor.matmul(ot[64:128, 0:NT],
                                 vt[b][1][0:NT2, h1s:h1s + HD],
                                 pj[0:NT2, 3, :], start=False, stop=True,
                                 tile_position=(0, 64))
                nc.tensor.matmul(ot[0:64, 256:256 + NT], ones_bf[:, :],
                                 pj[:, 0, :], start=True, stop=False)
                nc.tensor.matmul(ot[64:128, 256:256 + NT], ones_bf[:, :],
                                 pj[:, 2, :], start=True, stop=False,
                                 tile_position=(0, 64))
                nc.tensor.matmul(ot[0:64, 256:256 + NT], ones_bf[0:NT2, :],
                                 pj[0:NT2, 1, :], start=False, stop=True)
                nc.tensor.matmul(ot[64:128, 256:256 + NT], ones_bf[0:NT2, :],
                                 pj[0:NT2, 3, :], start=False, stop=True,
                                 tile_position=(0, 64))
                rc = rcp.tile([128, NT], F32, name="rc", tag="rc")
                nc.vector.reciprocal_approx_fast(
                    out=rc[:, :], in_=ot[:, 256:256 + NT])
                nc.vector.tensor_mul(
                    op_s[:, hp, b * NT:(b + 1) * NT],
                    ot[:, 0:NT], rc[:, :])

            # ---- projection group: one out-chunk c of token slice s ----
            def proj_group(c, s):
                t0, t1 = SLICES[s]
                acc = pb1.tile([128, SLW], F32, name="acc_p", tag="mm")
                for kp in range(KC):
                    nc.tensor.matmul(
                        acc[:, :],
                        pwt[kp][:, c * 128:(c + 1) * 128],
                        op_s[:, kp, t0:t1],
                        start=(kp == 0),
                        stop=(kp == KC - 1) and not proj_bias_nonzero,
                    )
                if proj_bias_nonzero:
                    nc.tensor.matmul(
                        acc[:, :],
                        pb_s[0:1, c * 128:(c + 1) * 128],
                        ones_bfr[0:1, :],
                        start=False, stop=True,
                    )
                obt = obp.tile([128, SLW], BF16, name="obt", tag="ob")
                nc.scalar.copy(obt[:, :], acc[:, :])
                nc.sync.dma_start(out[c * 128:(c + 1) * 128, t0:t1],
                                  obt[:, :])

            items = ([(hp, b) for b in range(BPC) for hp in range(HP)]
                     if PHASES in ("all", "sa") else [])
            pend = {}

            def do_stage_b(j):
                stage_b(*pend.pop(j))
                jhp, jb = items[j]
                # token slice s = batches (2s, 2s+1): emit its projection as
                # soon as the last norm of batch 2s+1 is issued, so output
                # DMA streams during attention instead of trailing it.
                if jhp == HP - 1 and jb % 2 == 1:
                    for c in range(KC):
                        proj_group(c, jb // 2)

            for i, (hp, b) in enumerate(items):
                pend[i] = (b, hp, stage_a(b, hp, i))
                if i >= SKEW and PHASES != "sa":
                    do_stage_b(i - SKEW)
            if PHASES != "sa":
                for i in sorted(pend):
                    do_stage_b(i)
            if PHASES != "all":
                for s in range(4):
                    for c in range(KC):
                        proj_group(c, s)

    nc.compile()
    return nc


@functools.lru_cache(maxsize=4)
def _built(qkv_bias_nonzero: bool, proj_bias_nonzero: bool):
    return build(qkv_bias_nonzero, proj_bias_nonzero)


def prepare_inputs(x, qkv_w, q_bias, v_bias, rpb_table, proj_w, proj_b, rel_index):
    """Host-side prep: shard + transpose + fold scale + gather bias table."""
    x = np.asarray(x, dtype=np.float32)
    qkv_w = np.asarray(qkv_w, dtype=np.float32)
    q_bias = np.asarray(q_bias, dtype=np.float32)
    v_bias = np.asarray(v_bias, dtype=np.float32)
    rpb_table = np.asarray(rpb_table, dtype=np.float32)
    proj_w = np.asarray(proj_w, dtype=np.float32)
    proj_b = np.asarray(proj_b, dtype=np.float32)
    rel_index = np.asarray(rel_index)

    qw = qkv_w[0:DIM] * np.float32(SCALE)   # exact: SCALE is a power of two
    qkw_fm = np.concatenate([qw, qkv_w[DIM:2 * DIM]], axis=0).T  # [768, 1536]
    # block to [c][feature-part p][kc, m] so DMA rows are 1536B contiguous
    qkw_h = np.ascontiguousarray(
        qkw_fm.reshape(KC, 128, 2 * KC, 128).transpose(2, 1, 0, 3)
        .reshape(2 * KC, 128, DIM)).astype(ml_dtypes.bfloat16)
    vw_h = np.ascontiguousarray(qkv_w[2 * DIM:3 * DIM].T).astype(
        ml_dtypes.bfloat16)                                      # [768, 768]
    pw_h = np.ascontiguousarray(proj_w.T).astype(ml_dtypes.bfloat16)

    # bias[i, j, h] -> biasT[h, j, i]
    bias = rpb_table[rel_index].astype(np.float32)               # (197,197,12)
    biasT = bias.transpose(2, 1, 0)                              # (12, j, i)
    ebT = np.exp(biasT)
    # expB for j1 (j=0:128), both heads of each pair adjacent
    # per-head quadrant table: [h][j-part][jchunk][i]; j2 rows 69:128 -> 1.0
    ebq_h = np.ones((H, 128, 2, NT), dtype=np.float32)
    for h in range(H):
        ebq_h[h, :, 0, :] = ebT[h, 0:128, :]
        ebq_h[h, 0:NT2, 1, :] = ebT[h, 128:NT, :]
    ebq_h = ebq_h.reshape(H, 128, 2 * NT).astype(ml_dtypes.bfloat16)
    ebb_h = np.zeros((HP, 128, 453), dtype=np.float32)
    for hp in range(HP):
        ebb_h[hp, 0:128, 0:NT] = biasT[2 * hp, 0:128, :]
        ebb_h[hp, 0:NT2, 256:256 + NT] = biasT[2 * hp, 128:NT, :]
    ebb_h = ebb_h.astype(ml_dtypes.bfloat16)
    id_h = np.eye(128, dtype=ml_dtypes.bfloat16)

    qkv_bias_nonzero = bool(q_bias.any() or v_bias.any())
    proj_bias_nonzero = bool(proj_b.any())

    in_maps = []
    for i in range(NCORES):
        xs = x[i * BPC:(i + 1) * BPC].reshape(TOK, DIM)
        m = {
            "xt": np.ascontiguousarray(xs.T).astype(ml_dtypes.bfloat16),
            "qkw": qkw_h, "vw": vw_h, "pw": pw_h,
            "ebq": ebq_h, "ebb": ebb_h, "idm": id_h,
        }
        if qkv_bias_nonzero:
            m["qkb"] = np.ascontiguousarray(
                np.concatenate([q_bias * np.float32(SCALE),
                                np.zeros_like(q_bias)])[None, :],
                dtype=np.float32).astype(ml_dtypes.bfloat16)
            m["vb"] = np.ascontiguousarray(
                v_bias[None, :]).astype(ml_dtypes.bfloat16)
        if proj_bias_nonzero:
            m["pb"] = np.ascontiguousarray(
                proj_b[None, :], dtype=np.float32).astype(ml_dtypes.bfloat16)
        in_maps.append(m)
    return in_maps, qkv_bias_nonzero, proj_bias_nonzero


def kernel(x, qkv_w, q_bias, v_bias, rpb_table, proj_w, proj_b, rel_index):
    in_maps, qb_nz, pb_nz = prepare_inputs(
        x, qkv_w, q_bias, v_bias, rpb_table, proj_w, proj_b, rel_index)
    nc = _built(qb_nz, pb_nz)
    res = run_bass_kernel_spmd(nc, in_maps, core_ids=list(range(NCORES)))
    outs = []
    for i in range(NCORES):
        ofm = res.results[i]["out"].astype(np.float32)    # [768, 1576]
        outs.append(ofm.T.reshape(BPC, NT, DIM))
    return np.concatenate(outs, axis=0).astype(np.float32)


# revision 35
# speedup vs baseline: 1.0055x; 1.0028x over previous
"""Trainium2 Bass kernel for windowed multi-head attention (nn_AttentionWindow).

Reference computation (B=64, N=197, DIM=768, H=12, HD=64):
    qkv  = x @ qkv_w.T + [q_bias, 0, v_bias]
    q, k, v = split(qkv);  q *= HD**-0.5
    attn = softmax(q @ k.T + rpb_table[rel_index])
    out  = (attn @ v) @ proj_w.T + proj_b

Sharding: data-parallel over batch across 8 NeuronCores (8 batches/core).

Per-core design (bf16 matmuls on TensorE, fp32 PSUM accumulation):
  - x pre-transposed on host to xT [768, 1576] bf16; weights host-blocked so
    every DMA is row-contiguous; per-chunk tiles let compute start earlier.
  - q,k feature-major into a resident [128, 12, 1576] tile, computed in
    4x394-token slices (no ragged tail matmuls).  v token-major per batch.
  - Attention per (head-pair, batch) item, software-pipelined (SKEW=3):
      * st PSUM quadrants [128,1024]: bank0 = head h0 (j1@0, j2@256, PE row
        position 0), bank1 = h1 (j1@512, j2@768, row position 64).  All
        matmul groups sharing a PSUM bank MUST share a PE row position.
      * bank0 is prefilled with h0's raw rel-pos bias via an identity
        matmul; h0 scores accumulate onto it (start=False), so the merged
        exp directly yields exp(S+B) for h0.  Softmax without max
        subtraction (scores are O(1): q pre-scaled by 1/8).
      * ONE merged ScalarE exp over all 4 quadrants -> P^T bf16.
      * h1's exp(bias) multiply runs on the otherwise-idle GpSimd engine.
      * O^T and denominators: col-packed matmuls (h0 -> PSUM partitions
        0:64, h1 -> 64:128 via tile_position(0,64); ones lhsT M=64 for the
        sums), so reciprocal and normalize are ONE [128,197] VectorE op
        each, writing proj's resident rhs [128, 6, 1576].
  - Projection feature-major; each 394-token slice is emitted as soon as
    its two batches finish attention, so output DMA overlaps compute.
    Output bf16 [768, 1576]; host transposes and upcasts.
"""
import sys
import functools

sys.path.insert(0, "/opt/trn_rl_repo")

import numpy as np
import ml_dtypes

import concourse.bass as bass  # noqa: E402
import concourse.bacc as bacc  # noqa: E402
import concourse.mybir as mybir  # noqa: E402
from concourse.tile import TileContext  # noqa: E402
from concourse.bass_utils import run_bass_kernel_spmd  # noqa: E402

F32 = mybir.dt.float32
BF16 = mybir.dt.bfloat16

NCORES = 8
B, NT, DIM = 64, 197, 768
H, HD = 12, 64
SCALE = HD ** -0.5  # 0.125, exact power of two -> folded into q weights
BPC = B // NCORES   # 8 batches per core
TOK = BPC * NT      # 1576 tokens per core
KC = DIM // 128     # 6
HP = H // 2         # 6 head pairs
NT2 = NT - 128      # 69 (second token chunk)
SKEW = 3            # attention software-pipeline depth (items)
SLW = 394           # token slice width (4 * 394 = 1576 exactly)
SLICES = [(s * SLW, (s + 1) * SLW) for s in range(4)]
MERGED_EXP = True   # one exp over all 4 score quadrants
EB_ENGINE = "gpsimd"  # engine for the j1 expB multiply: "vector"|"gpsimd"
PHASES = "all"
V_FM = False    # v feature-major + XBAR DMA transpose to token-major      # debug: "qkproj" | "qkv" | "all"


def build(qkv_bias_nonzero: bool, proj_bias_nonzero: bool):
    nc = bacc.Bacc("TRN2", target_bir_lowering=False, debug=False)

    xt = nc.dram_tensor("xt", [DIM, TOK], BF16, kind="ExternalInput")
    # host-blocked per out-chunk: [c][feature-part][kc*128+m], contiguous rows
    qkw = nc.dram_tensor("qkw", [2 * KC, 128, DIM], BF16, kind="ExternalInput")
    vw = nc.dram_tensor("vw", [DIM, DIM], BF16, kind="ExternalInput")
    pw = nc.dram_tensor("pw", [DIM, DIM], BF16, kind="ExternalInput")
    # expB per head: [h][j=0:128][jchunk][i]; jchunk 1 rows 69:128 = 1.0
    ebq = nc.dram_tensor("ebq", [H, 128, 2 * NT], BF16, kind="ExternalInput")
    # raw bias for even heads, bank-0 quadrant layout [hp][128][453]
    ebb = nc.dram_tensor("ebb", [HP, 128, 453], BF16, kind="ExternalInput")
    idm = nc.dram_tensor("idm", [128, 128], BF16, kind="ExternalInput")
    out = nc.dram_tensor("out", [DIM, TOK], BF16, kind="ExternalOutput")
    if qkv_bias_nonzero:
        qkb = nc.dram_tensor("qkb", [1, 2 * DIM], BF16, kind="ExternalInput")
        vb = nc.dram_tensor("vb", [1, DIM], BF16, kind="ExternalInput")
    if proj_bias_nonzero:
        pb = nc.dram_tensor("pb", [1, DIM], BF16, kind="ExternalInput")

    with TileContext(nc) as tc:
        with (
            tc.tile_pool(name="const", bufs=1) as constp,
            tc.tile_pool(name="vp", bufs=2 * BPC) as vp,
            tc.tile_pool(name="pp", bufs=SKEW + 2) as pp,
            tc.tile_pool(name="rcp", bufs=3) as rcp,
            tc.tile_pool(name="obp", bufs=4) as obp,
            tc.tile_pool(name="pb1", bufs=4, space="PSUM") as pb1,
            tc.tile_pool(name="sta", bufs=2, space="PSUM") as sta,
        ):
            # ---- resident inputs ----
            # DMA rings round-robin by issue index; split the inputs into
            # pieces issued in need-order so the first qk groups' data gets
            # the full aggregate bandwidth up front.
            xbt = [constp.tile([128, TOK], BF16, name=f"xb{kc}")
                   for kc in range(KC)]
            qkwc = [constp.tile([128, KC, 128], BF16, name=f"qkw{c}")
                    for c in range(2 * KC)]
            vwt = [constp.tile([128, DIM], BF16, name=f"vw{kc}")
                   for kc in range(KC)]
            pwt = [constp.tile([128, DIM], BF16, name=f"pw{kc}")
                   for kc in range(KC)]

            for kc in range(KC):
                nc.sync.dma_start(xbt[kc][:, :], xt[kc * 128:(kc + 1) * 128, :])
            for c in range(2 * KC):
                nc.sync.dma_start(
                    qkwc[c][:, :, :],
                    qkw[c, :, :].rearrange("p (k m) -> p k m", k=KC))
            for kc in range(KC):
                nc.sync.dma_start(vwt[kc][:, :], vw[kc * 128:(kc + 1) * 128, :])
            for kc in range(KC):
                nc.sync.dma_start(pwt[kc][:, :], pw[kc * 128:(kc + 1) * 128, :])
            ebq_s = {}
            for h in range(1, H, 2):
                ebq_s[h] = constp.tile([128, 2 * NT], BF16, name=f"ebq{h}")
                nc.sync.dma_start(ebq_s[h][:, :], ebq[h, :, :])
            ebb_s = [constp.tile([128, 453], BF16, name=f"ebb{hp}")
                     for hp in range(HP)]
            for hp in range(HP):
                nc.sync.dma_start(ebb_s[hp][:, :], ebb[hp, :, :])
            id_s = constp.tile([128, 128], BF16, name="id_s")
            nc.sync.dma_start(id_s[:, :], idm[:, :])
            ones_bf = constp.tile([128, 64], BF16, name="ones_bf")
            nc.gpsimd.memset(ones_bf[:, :], 1.0)
            if qkv_bias_nonzero:
                qkb_s = constp.tile([1, 2 * DIM], BF16, name="qkb_s")
                vb_s = constp.tile([1, DIM], BF16, name="vb_s")
                nc.sync.dma_start(qkb_s[:, :], qkb[:, :])
                nc.sync.dma_start(vb_s[:, :], vb[:, :])
            if proj_bias_nonzero:
                pb_s = constp.tile([1, DIM], BF16, name="pb_s")
                nc.sync.dma_start(pb_s[:, :], pb[:, :])
            if qkv_bias_nonzero or proj_bias_nonzero:
                ones_bfr = constp.tile([1, SLW], BF16, name="ones_bfr")
                nc.gpsimd.memset(ones_bfr[:, :], 1.0)
            # big resident activations: q,k and proj-rhs (bf16)
            qk_s = constp.tile([128, 2 * KC, TOK], BF16, name="qk_s")
            op_s = constp.tile([128, KC, TOK], BF16, name="op_s")

            # ---- q,k feature-major: 12 channel-chunks x 4 token slices ----
            for c in range(2 * KC):
                for t0, t1 in SLICES:
                    acc = pb1.tile([128, SLW], F32, name="acc_qk", tag="mm")
                    for kc in range(KC):
                        nc.tensor.matmul(
                            acc[:, :],
                            qkwc[c][:, kc, :],
                            xbt[kc][:, t0:t1],
                            start=(kc == 0),
                            stop=(kc == KC - 1) and not qkv_bias_nonzero,
                        )
                    if qkv_bias_nonzero:
                        nc.tensor.matmul(
                            acc[:, :],
                            qkb_s[0:1, c * 128:(c + 1) * 128],
                            ones_bfr[0:1, :],
                            start=False, stop=True,
                        )
                    nc.scalar.copy(qk_s[:, c, t0:t1], acc[:, :])

            # ---- v token-major per batch: [(128|69) tok, 768 ch] ----
            vt = [[None, None] for _ in range(BPC)]
            for b in range(BPC):
                for tch in range(2):
                    toff = b * NT + tch * 128
                    tlen = 128 if tch == 0 else NT2
                    t = vp.tile([128, DIM], BF16, name="v_t", tag="v")
                    for half in range(2):
                        n0, n1 = half * 384, (half + 1) * 384
                        acc = pb1.tile([128, 384], F32, name="acc_v", tag="mm")
                        for kc in range(KC):
                            nc.tensor.matmul(
                                acc[0:tlen, :],
                                xbt[kc][:, toff:toff + tlen],
                                vwt[kc][:, n0:n1],
                                start=(kc == 0),
                                stop=(kc == KC - 1) and not qkv_bias_nonzero,
                            )
                        if qkv_bias_nonzero:
                            nc.tensor.matmul(
                                acc[0:tlen, :],
                                ones_bfr[0:1, 0:tlen],
                                vb_s[0:1, n0:n1],
                                start=False, stop=True,
                            )
                        nc.vector.tensor_copy(t[0:tlen, n0:n1],
                                              acc[0:tlen, :])
                    vt[b][tch] = t

            # ---- attention, software-pipelined over (head-pair, batch) ----
            # st quadrants: [0:197]=j1h0 [256:453]=j1h1 [512:709]=j2h0
            # [768:965]=j2h1 (j2 quadrants pre-filled with raw bias).
            eb_eng = nc.gpsimd if EB_ENGINE == "gpsimd" else nc.vector

            def stage_a(b, hp):
                st = sta.tile([128, 1024], F32, name="st", tag="sta")
                q0 = qk_s[0:64, hp, b * NT:(b + 1) * NT]
                q1 = qk_s[64:128, hp, b * NT:(b + 1) * NT]
                # quadrants: [0:197]=j1h0 [256:453]=j2h0 (bank 0, row pos 0)
                #            [512:709]=j1h1 [768:965]=j2h1 (bank 1, row pos 64)
                # bank 0 is prefilled with h0's raw bias (identity matmul, row
                # pos 0 like the h0 scores); h0 scores accumulate onto it so
                # exp(S+B) comes straight out of the activation.
                nc.tensor.matmul(st[:, 0:453], id_s[:, :], ebb_s[hp][:, :],
                                 start=True, stop=False)
                nc.tensor.matmul(st[:, 0:NT],
                                 qk_s[0:64, KC + hp, b * NT:b * NT + 128],
                                 q0, start=False, stop=True)
                nc.tensor.matmul(st[:, 512:512 + NT],
                                 qk_s[64:128, KC + hp, b * NT:b * NT + 128],
                                 q1, start=True, stop=True)
                nc.tensor.matmul(st[0:NT2, 256:256 + NT],
                                 qk_s[0:64, KC + hp, b * NT + 128:(b + 1) * NT],
                                 q0, start=False, stop=True)
                nc.tensor.matmul(st[0:NT2, 768:768 + NT],
                                 qk_s[64:128, KC + hp, b * NT + 128:(b + 1) * NT],
                                 q1, start=True, stop=True)
                pj = pp.tile([128, 4, NT], BF16, name="pj", tag="p")
                # one exp over all 4 quadrants (c = j1h0, j2h0, j1h1, j2h1)
                nc.scalar.activation(
                    pj[:, :, :],
                    st[:, :].rearrange("p (c x) -> p c x", c=4)[:, :, 0:NT],
                    mybir.ActivationFunctionType.Exp)
                # h1 expB multiply on gpsimd (h0 bias was added in PSUM)
                pjf = pj[:, :, :].rearrange("p c x -> p (c x)")
                nc.gpsimd.tensor_mul(pjf[:, 2 * NT:4 * NT],
                                     pjf[:, 2 * NT:4 * NT],
                                     ebq_s[2 * hp + 1][:, :])
                return pj

            def stage_b(b, hp, pj):
                """O^T + denominators col-packed (h0->rows 0:64, h1->64:128),
                one reciprocal + one normalize on VectorE."""
                ot = pb1.tile([128, 512], F32, name="ot", tag="mm")
                h0s, h1s = 2 * hp * HD, (2 * hp + 1) * HD
                nc.tensor.matmul(ot[0:64, 0:NT],
                                 vt[b][0][:, h0s:h0s + HD],
                                 pj[:, 0, :], start=True, stop=False)
                nc.tensor.matmul(ot[64:128, 0:NT],
                                 vt[b][0][:, h1s:h1s + HD],
                                 pj[:, 2, :], start=True, stop=False,
                                 tile_position=(0, 64))
                nc.tensor.matmul(ot[0:64, 0:NT],
                                 vt[b][1][0:NT2, h0s:h0s + HD],
                                 pj[0:NT2, 1, :], start=False, stop=True)
                nc.tensor.matmul(ot[64:128, 0:NT],
                                 vt[b][1][0:NT2, h1s:h1s + HD],
                                 pj[0:NT2, 3, :], start=False, stop=True,
                                 tile_position=(0, 64))
                nc.tensor.matmul(ot[0:64, 256:256 + NT], ones_bf[:, :],
                                 pj[:, 0, :], start=True, stop=False)
                nc.tensor.matmul(ot[64:128, 256:256 + NT], ones_bf[:, :],
                                 pj[:, 2, :], start=True, stop=False,
                                 tile_position=(0, 64))
                nc.tensor.matmul(ot[0:64, 256:256 + NT], ones_bf[0:NT2, :],
                                 pj[0:NT2, 1, :], start=False, stop=True)
                nc.tensor.matmul(ot[64:128, 256:256 + NT], ones_bf[0:NT2, :],
                                 pj[0:NT2, 3, :], start=False, stop=True,
                                 tile_position=(0, 64))
                rc = rcp.tile([128, NT], F32, name="rc", tag="rc")
                nc.vector.reciprocal_approx_fast(
                    out=rc[:, :], in_=ot[:, 256:256 + NT])
                nc.vector.tensor_mul(
                    op_s[:, hp, b * NT:(b + 1) * NT],
                    ot[:, 0:NT], rc[:, :])

            # ---- projection group: one out-chunk c of token slice s ----
            def proj_group(c, s):
                t0, t1 = SLICES[s]
                acc = pb1.tile([128, SLW], F32, name="acc_p", tag="mm")
                for kp in range(KC):
                    nc.tensor.matmul(
                        acc[:, :],
                        pwt[kp][:, c * 128:(c + 1) * 128],
                        op_s[:, kp, t0:t1],
                        start=(kp == 0),
                        stop=(kp == KC - 1) and not proj_bias_nonzero,
                    )
                if proj_bias_nonzero:
                    nc.tensor.matmul(
                        acc[:, :],
                        pb_s[0:1, c * 128:(c + 1) * 128],
                        ones_bfr[0:1, :],
                        start=False, stop=True,
                    )
                obt = obp.tile([128, SLW], BF16, name="obt", tag="ob")
                nc.scalar.copy(obt[:, :], acc[:, :])
                nc.sync.dma_start(out[c * 128:(c + 1) * 128, t0:t1],
                                  obt[:, :])

            items = [(hp, b) for b in range(BPC) for hp in range(HP)]
            pend = {}

            def do_stage_b(j):
                stage_b(*pend.pop(j))
                jhp, jb = items[j]
                # token slice s = batches (2s, 2s+1): emit its projection as
                # soon as the last norm of batch 2s+1 is issued, so output
                # DMA streams during attention instead of trailing it.
                if jhp == HP - 1 and jb % 2 == 1:
                    for c in range(KC):
                        proj_group(c, jb // 2)

            for i, (hp, b) in enumerate(items):
                pend[i] = (b, hp, stage_a(b, hp))
                if i >= SKEW:
                    do_stage_b(i - SKEW)
            for i in sorted(pend):
                do_stage_b(i)

    nc.compile()
    return nc


@functools.lru_cache(maxsize=4)
def _built(qkv_bias_nonzero: bool, proj_bias_nonzero: bool):
    return build(qkv_bias_nonzero, proj_bias_nonzero)


def prepare_inputs(x, qkv_w, q_bias, v_bias, rpb_table, proj_w, proj_b, rel_index):
    """Host-side prep: shard + transpose + fold scale + gather bias table."""
    x = np.asarray(x, dtype=np.float32)
    qkv_w = np.asarray(qkv_w, dtype=np.float32)
    q_bias = np.asarray(q_bias, dtype=np.float32)
    v_bias = np.asarray(v_bias, dtype=np.float32)
    rpb_table = np.asarray(rpb_table, dtype=np.float32)
    proj_w = np.asarray(proj_w, dtype=np.float32)
    proj_b = np.asarray(proj_b, dtype=np.float32)
    rel_index = np.asarray(rel_index)

    qw = qkv_w[0:DIM] * np.float32(SCALE)   # exact: SCALE is a power of two
    qkw_fm = np.concatenate([qw, qkv_w[DIM:2 * DIM]], axis=0).T  # [768, 1536]
    # block to [c][feature-part p][kc, m] so DMA rows are 1536B contiguous
    qkw_h = np.ascontiguousarray(
        qkw_fm.reshape(KC, 128, 2 * KC, 128).transpose(2, 1, 0, 3)
        .reshape(2 * KC, 128, DIM)).astype(ml_dtypes.bfloat16)
    vw_h = np.ascontiguousarray(qkv_w[2 * DIM:3 * DIM].T).astype(
        ml_dtypes.bfloat16)                                      # [768, 768]
    pw_h = np.ascontiguousarray(proj_w.T).astype(ml_dtypes.bfloat16)

    # bias[i, j, h] -> biasT[h, j, i]
    bias = rpb_table[rel_index].astype(np.float32)               # (197,197,12)
    biasT = bias.transpose(2, 1, 0)                              # (12, j, i)
    ebT = np.exp(biasT)
    # expB for j1 (j=0:128), both heads of each pair adjacent
    # per-head quadrant table: [h][j-part][jchunk][i]; j2 rows 69:128 -> 1.0
    ebq_h = np.ones((H, 128, 2, NT), dtype=np.float32)
    for h in range(H):
        ebq_h[h, :, 0, :] = ebT[h, 0:128, :]
        ebq_h[h, 0:NT2, 1, :] = ebT[h, 128:NT, :]
    ebq_h = ebq_h.reshape(H, 128, 2 * NT).astype(ml_dtypes.bfloat16)
    ebb_h = np.zeros((HP, 128, 453), dtype=np.float32)
    for hp in range(HP):
        ebb_h[hp, 0:128, 0:NT] = biasT[2 * hp, 0:128, :]
        ebb_h[hp, 0:NT2, 256:256 + NT] = biasT[2 * hp, 128:NT, :]
    ebb_h = ebb_h.astype(ml_dtypes.bfloat16)
    id_h = np.eye(128, dtype=ml_dtypes.bfloat16)

    qkv_bias_nonzero = bool(q_bias.any() or v_bias.any())
    proj_bias_nonzero = bool(proj_b.any())

    in_maps = []
    for i in range(NCORES):
        xs = x[i * BPC:(i + 1) * BPC].reshape(TOK, DIM)
        m = {
            "xt": np.ascontiguousarray(xs.T).astype(ml_dtypes.bfloat16),
            "qkw": qkw_h, "vw": vw_h, "pw": pw_h,
            "ebq": ebq_h, "ebb": ebb_h, "idm": id_h,
        }
        if qkv_bias_nonzero:
            m["qkb"] = np.ascontiguousarray(
                np.concatenate([q_bias * np.float32(SCALE),
                                np.zeros_like(q_bias)])[None, :],
                dtype=np.float32).astype(ml_dtypes.bfloat16)
            m["vb"] = np.ascontiguousarray(
                v_bias[None, :]).astype(ml_dtypes.bfloat16)
        if proj_bias_nonzero:
            m["pb"] = np.ascontiguousarray(
                proj_b[None, :], dtype=np.float32).astype(ml_dtypes.bfloat16)
        in_maps.append(m)
    return in_maps, qkv_bias_nonzero, proj_bias_nonzero


def kernel(x, qkv_w, q_bias, v_bias, rpb_table, proj_w, proj_b, rel_index):
    in_maps, qb_nz, pb_nz = prepare_inputs(
        x, qkv_w, q_bias, v_bias, rpb_table, proj_w, proj_b, rel_index)
    nc = _built(qb_nz, pb_nz)
    res = run_bass_kernel_spmd(nc, in_maps, core_ids=list(range(NCORES)))
    outs = []
    for i in range(NCORES):
        ofm = res.results[i]["out"].astype(np.float32)    # [768, 1576]
        outs.append(ofm.T.reshape(BPC, NT, DIM))
    return np.concatenate(outs, axis=0).astype(np.float32)


# revision 36
# speedup vs baseline: 1.1948x; 1.1882x over previous
"""Trainium2 Bass kernel for windowed multi-head attention (nn_AttentionWindow).

Reference computation (B=64, N=197, DIM=768, H=12, HD=64):
    qkv  = x @ qkv_w.T + [q_bias, 0, v_bias]
    q, k, v = split(qkv);  q *= HD**-0.5
    attn = softmax(q @ k.T + rpb_table[rel_index])
    out  = (attn @ v) @ proj_w.T + proj_b

Sharding: data-parallel over batch across 8 NeuronCores (8 batches/core).

Per-core design (bf16 matmuls on TensorE, fp32 PSUM accumulation):
  - x pre-transposed on host to xT [768, 1576] bf16; weights host-blocked so
    every DMA is row-contiguous; per-chunk tiles let compute start earlier.
  - q,k feature-major into a resident [128, 12, 1576] tile, computed in
    4x394-token slices (no ragged tail matmuls).  v token-major per batch.
  - Attention per (head-pair, batch) item, software-pipelined (SKEW=3):
      * st PSUM quadrants [128,1024]: bank0 = head h0 (j1@0, j2@256, PE row
        position 0), bank1 = h1 (j1@512, j2@768, row position 64).  All
        matmul groups sharing a PSUM bank MUST share a PE row position.
      * bank0 is prefilled with h0's raw rel-pos bias via an identity
        matmul; h0 scores accumulate onto it (start=False), so the merged
        exp directly yields exp(S+B) for h0.  Softmax without max
        subtraction (scores are O(1): q pre-scaled by 1/8).
      * ONE merged ScalarE exp over all 4 quadrants -> P^T bf16.
      * h1's exp(bias) multiply runs on the otherwise-idle GpSimd engine.
      * O^T and denominators: col-packed matmuls (h0 -> PSUM partitions
        0:64, h1 -> 64:128 via tile_position(0,64); ones lhsT M=64 for the
        sums), so reciprocal and normalize are ONE [128,197] VectorE op
        each, writing proj's resident rhs [128, 6, 1576].
  - Projection feature-major; each 394-token slice is emitted as soon as
    its two batches finish attention, so output DMA overlaps compute.
    Output bf16 [768, 1576]; host transposes and upcasts.
"""
import sys
import functools

sys.path.insert(0, "/opt/trn_rl_repo")

import numpy as np
import ml_dtypes

import concourse.bass as bass  # noqa: E402
import concourse.bacc as bacc  # noqa: E402
import concourse.mybir as mybir  # noqa: E402
from concourse.tile import TileContext  # noqa: E402
from concourse.bass_utils import run_bass_kernel_spmd  # noqa: E402

F32 = mybir.dt.float32
BF16 = mybir.dt.bfloat16

NCORES = 8
B, NT, DIM = 64, 197, 768
H, HD = 12, 64
SCALE = HD ** -0.5  # 0.125, exact power of two -> folded into q weights
BPC = B // NCORES   # 8 batches per core
TOK = BPC * NT      # 1576 tokens per core
KC = DIM // 128     # 6
HP = H // 2         # 6 head pairs
NT2 = NT - 128      # 69 (second token chunk)
SKEW = 3            # attention software-pipeline depth (items)
SLW = 394           # token slice width (4 * 394 = 1576 exactly)
SLICES = [(s * SLW, (s + 1) * SLW) for s in range(4)]


def build(qkv_bias_nonzero: bool, proj_bias_nonzero: bool):
    nc = bacc.Bacc("TRN2", target_bir_lowering=False, debug=False)

    xt = nc.dram_tensor("xt", [DIM, TOK], BF16, kind="ExternalInput")
    # host-blocked per out-chunk: [c][feature-part][kc*128+m], contiguous rows
    qkw = nc.dram_tensor("qkw", [2 * KC, 128, DIM], BF16, kind="ExternalInput")
    vw = nc.dram_tensor("vw", [DIM, DIM], BF16, kind="ExternalInput")
    pw = nc.dram_tensor("pw", [DIM, DIM], BF16, kind="ExternalInput")
    # expB per head: [h][j=0:128][jchunk][i]; jchunk 1 rows 69:128 = 1.0
    ebq = nc.dram_tensor("ebq", [H, 128, 2 * NT], BF16, kind="ExternalInput")
    # raw bias for even heads, bank-0 quadrant layout [hp][128][453]
    ebb = nc.dram_tensor("ebb", [HP, 128, 453], BF16, kind="ExternalInput")
    idm = nc.dram_tensor("idm", [128, 128], BF16, kind="ExternalInput")
    out = nc.dram_tensor("out", [DIM, TOK], BF16, kind="ExternalOutput")
    if qkv_bias_nonzero:
        qkb = nc.dram_tensor("qkb", [1, 2 * DIM], BF16, kind="ExternalInput")
        vb = nc.dram_tensor("vb", [1, DIM], BF16, kind="ExternalInput")
    if proj_bias_nonzero:
        pb = nc.dram_tensor("pb", [1, DIM], BF16, kind="ExternalInput")

    with TileContext(nc) as tc:
        with (
            tc.tile_pool(name="const", bufs=1) as constp,
            tc.tile_pool(name="vp", bufs=2 * BPC) as vp,
            tc.tile_pool(name="pp", bufs=SKEW + 2) as pp,
            tc.tile_pool(name="rcp", bufs=3) as rcp,
            tc.tile_pool(name="obp", bufs=4) as obp,
            tc.tile_pool(name="pb1", bufs=4, space="PSUM") as pb1,
            tc.tile_pool(name="sta", bufs=2, space="PSUM") as sta,
        ):
            # ---- resident inputs ----
            # DMA rings round-robin by issue index; split the inputs into
            # pieces issued in need-order so the first qk groups' data gets
            # the full aggregate bandwidth up front.
            xbt = [constp.tile([128, TOK], BF16, name=f"xb{kc}")
                   for kc in range(KC)]
            qkwc = [constp.tile([128, KC, 128], BF16, name=f"qkw{c}")
                    for c in range(2 * KC)]
            vwt = [constp.tile([128, DIM], BF16, name=f"vw{kc}")
                   for kc in range(KC)]
            pwt = [constp.tile([128, DIM], BF16, name=f"pw{kc}")
                   for kc in range(KC)]

            for kc in range(KC):
                nc.sync.dma_start(xbt[kc][:, :], xt[kc * 128:(kc + 1) * 128, :])
            for c in range(2 * KC):
                nc.sync.dma_start(
                    qkwc[c][:, :, :],
                    qkw[c, :, :].rearrange("p (k m) -> p k m", k=KC))
            for kc in range(KC):
                nc.sync.dma_start(vwt[kc][:, :], vw[kc * 128:(kc + 1) * 128, :])
            for kc in range(KC):
                nc.sync.dma_start(pwt[kc][:, :], pw[kc * 128:(kc + 1) * 128, :])
            ebq_s = {}
            for h in range(1, H, 2):
                ebq_s[h] = constp.tile([128, 2 * NT], BF16, name=f"ebq{h}")
                nc.sync.dma_start(ebq_s[h][:, :], ebq[h, :, :])
            ebb_s = [constp.tile([128, 453], BF16, name=f"ebb{hp}")
                     for hp in range(HP)]
            for hp in range(HP):
                nc.sync.dma_start(ebb_s[hp][:, :], ebb[hp, :, :])
            id_s = constp.tile([128, 128], BF16, name="id_s")
            nc.sync.dma_start(id_s[:, :], idm[:, :])
            ones_bf = constp.tile([128, 64], BF16, name="ones_bf")
            nc.gpsimd.memset(ones_bf[:, :], 1.0)
            if qkv_bias_nonzero:
                qkb_s = constp.tile([1, 2 * DIM], BF16, name="qkb_s")
                vb_s = constp.tile([1, DIM], BF16, name="vb_s")
                nc.sync.dma_start(qkb_s[:, :], qkb[:, :])
                nc.sync.dma_start(vb_s[:, :], vb[:, :])
            if proj_bias_nonzero:
                pb_s = constp.tile([1, DIM], BF16, name="pb_s")
                nc.sync.dma_start(pb_s[:, :], pb[:, :])
            if qkv_bias_nonzero or proj_bias_nonzero:
                ones_bfr = constp.tile([1, SLW], BF16, name="ones_bfr")
                nc.gpsimd.memset(ones_bfr[:, :], 1.0)
            # big resident activations: q,k and proj-rhs (bf16)
            qk_s = constp.tile([128, 2 * KC, TOK], BF16, name="qk_s")
            op_s = constp.tile([128, KC, TOK], BF16, name="op_s")

            # ---- q,k feature-major: 12 channel-chunks x 4 token slices ----
            for c in range(2 * KC):
                for t0, t1 in SLICES:
                    acc = pb1.tile([128, SLW], F32, name="acc_qk", tag="mm")
                    for kc in range(KC):
                        nc.tensor.matmul(
                            acc[:, :],
                            qkwc[c][:, kc, :],
                            xbt[kc][:, t0:t1],
                            start=(kc == 0),
                            stop=(kc == KC - 1) and not qkv_bias_nonzero,
                        )
                    if qkv_bias_nonzero:
                        nc.tensor.matmul(
                            acc[:, :],
                            qkb_s[0:1, c * 128:(c + 1) * 128],
                            ones_bfr[0:1, :],
                            start=False, stop=True,
                        )
                    nc.scalar.copy(qk_s[:, c, t0:t1], acc[:, :])

            # ---- v token-major per batch: [(128|69) tok, 768 ch] ----
            vt = [[None, None] for _ in range(BPC)]
            for b in range(BPC):
                for tch in range(2):
                    toff = b * NT + tch * 128
                    tlen = 128 if tch == 0 else NT2
                    t = vp.tile([128, DIM], BF16, name="v_t", tag="v")
                    for half in range(2):
                        n0, n1 = half * 384, (half + 1) * 384
                        acc = pb1.tile([128, 384], F32, name="acc_v", tag="mm")
                        for kc in range(KC):
                            nc.tensor.matmul(
                                acc[0:tlen, :],
                                xbt[kc][:, toff:toff + tlen],
                                vwt[kc][:, n0:n1],
                                start=(kc == 0),
                                stop=(kc == KC - 1) and not qkv_bias_nonzero,
                            )
                        if qkv_bias_nonzero:
                            nc.tensor.matmul(
                                acc[0:tlen, :],
                                ones_bfr[0:1, 0:tlen],
                                vb_s[0:1, n0:n1],
                                start=False, stop=True,
                            )
                        nc.vector.tensor_copy(t[0:tlen, n0:n1],
                                              acc[0:tlen, :])
                    vt[b][tch] = t

            # ---- attention, software-pipelined over (head-pair, batch) ----
            # st quadrants: [0:197]=j1h0 [256:453]=j1h1 [512:709]=j2h0
            # [768:965]=j2h1 (j2 quadrants pre-filled with raw bias).
            def stage_a(b, hp):
                st = sta.tile([128, 1024], F32, name="st", tag="sta")
                q0 = qk_s[0:64, hp, b * NT:(b + 1) * NT]
                q1 = qk_s[64:128, hp, b * NT:(b + 1) * NT]
                # quadrants: [0:197]=j1h0 [256:453]=j2h0 (bank 0, row pos 0)
                #            [512:709]=j1h1 [768:965]=j2h1 (bank 1, row pos 64)
                # bank 0 is prefilled with h0's raw bias (identity matmul, row
                # pos 0 like the h0 scores); h0 scores accumulate onto it so
                # exp(S+B) comes straight out of the activation.
                nc.tensor.matmul(st[:, 0:453], id_s[:, :], ebb_s[hp][:, :],
                                 start=True, stop=False)
                nc.tensor.matmul(st[:, 0:NT],
                                 qk_s[0:64, KC + hp, b * NT:b * NT + 128],
                                 q0, start=False, stop=True)
                nc.tensor.matmul(st[:, 512:512 + NT],
                                 qk_s[64:128, KC + hp, b * NT:b * NT + 128],
                                 q1, start=True, stop=True)
                nc.tensor.matmul(st[0:NT2, 256:256 + NT],
                                 qk_s[0:64, KC + hp, b * NT + 128:(b + 1) * NT],
                                 q0, start=False, stop=True)
                nc.tensor.matmul(st[0:NT2, 768:768 + NT],
                                 qk_s[64:128, KC + hp, b * NT + 128:(b + 1) * NT],
                                 q1, start=True, stop=True)
                pj = pp.tile([128, 4, NT], BF16, name="pj", tag="p")
                # one exp over all 4 quadrants (c = j1h0, j2h0, j1h1, j2h1)
                nc.scalar.activation(
                    pj[:, :, :],
                    st[:, :].rearrange("p (c x) -> p c x", c=4)[:, :, 0:NT],
                    mybir.ActivationFunctionType.Exp)
                # h1 expB multiply on gpsimd (h0 bias was added in PSUM)
                pjf = pj[:, :, :].rearrange("p c x -> p (c x)")
                nc.gpsimd.tensor_mul(pjf[:, 2 * NT:4 * NT],
                                     pjf[:, 2 * NT:4 * NT],
                                     ebq_s[2 * hp + 1][:, :])
                return pj

            def stage_b(b, hp, pj):
                """O^T + denominators col-packed (h0->rows 0:64, h1->64:128),
                one reciprocal + one normalize on VectorE."""
                ot = pb1.tile([128, 512], F32, name="ot", tag="mm")
                h0s, h1s = 2 * hp * HD, (2 * hp + 1) * HD
                nc.tensor.matmul(ot[0:64, 0:NT],
                                 vt[b][0][:, h0s:h0s + HD],
                                 pj[:, 0, :], start=True, stop=False)
                nc.tensor.matmul(ot[64:128, 0:NT],
                                 vt[b][0][:, h1s:h1s + HD],
                                 pj[:, 2, :], start=True, stop=False,
                                 tile_position=(0, 64))
                nc.tensor.matmul(ot[0:64, 0:NT],
                                 vt[b][1][0:NT2, h0s:h0s + HD],
                                 pj[0:NT2, 1, :], start=False, stop=True)
                nc.tensor.matmul(ot[64:128, 0:NT],
                                 vt[b][1][0:NT2, h1s:h1s + HD],
                                 pj[0:NT2, 3, :], start=False, stop=True,
                                 tile_position=(0, 64))
                nc.tensor.matmul(ot[0:64, 256:256 + NT], ones_bf[:, :],
                                 pj[:, 0, :], start=True, stop=False)
                nc.tensor.matmul(ot[64:128, 256:256 + NT], ones_bf[:, :],
                                 pj[:, 2, :], start=True, stop=False,
                                 tile_position=(0, 64))
                nc.tensor.matmul(ot[0:64, 256:256 + NT], ones_bf[0:NT2, :],
                                 pj[0:NT2, 1, :], start=False, stop=True)
                nc.tensor.matmul(ot[64:128, 256:256 + NT], ones_bf[0:NT2, :],
                                 pj[0:NT2, 3, :], start=False, stop=True,
                                 tile_position=(0, 64))
                rc = rcp.tile([128, NT], F32, name="rc", tag="rc")
                nc.vector.reciprocal_approx_fast(
                    out=rc[:, :], in_=ot[:, 256:256 + NT])
                nc.vector.tensor_mul(
                    op_s[:, hp, b * NT:(b + 1) * NT],
                    ot[:, 0:NT], rc[:, :])

            # ---- projection group: one out-chunk c of token slice s ----
            def proj_group(c, s):
                t0, t1 = SLICES[s]
                acc = pb1.tile([128, SLW], F32, name="acc_p", tag="mm")
                for kp in range(KC):
                    nc.tensor.matmul(
                        acc[:, :],
                        pwt[kp][:, c * 128:(c + 1) * 128],
                        op_s[:, kp, t0:t1],
                        start=(kp == 0),
                        stop=(kp == KC - 1) and not proj_bias_nonzero,
                    )
                if proj_bias_nonzero:
                    nc.tensor.matmul(
                        acc[:, :],
                        pb_s[0:1, c * 128:(c + 1) * 128],
                        ones_bfr[0:1, :],
                        start=False, stop=True,
                    )
                obt = obp.tile([128, SLW], BF16, name="obt", tag="ob")
                nc.scalar.copy(obt[:, :], acc[:, :])
                nc.sync.dma_start(out[c * 128:(c + 1) * 128, t0:t1],
                                  obt[:, :])

            items = [(hp, b) for b in range(BPC) for hp in range(HP)]
            pend = {}

            def do_stage_b(j):
                stage_b(*pend.pop(j))
                jhp, jb = items[j]
                # token slice s = batches (2s, 2s+1): emit its projection as
                # soon as the last norm of batch 2s+1 is issued, so output
                # DMA streams during attention instead of trailing it.
                if jhp == HP - 1 and jb % 2 == 1:
                    for c in range(KC):
                        proj_group(c, jb // 2)

            for i, (hp, b) in enumerate(items):
                pend[i] = (b, hp, stage_a(b, hp))
                if i >= SKEW:
                    do_stage_b(i - SKEW)
            for i in sorted(pend):
                do_stage_b(i)

    nc.compile()
    return nc


@functools.lru_cache(maxsize=4)
def _built(qkv_bias_nonzero: bool, proj_bias_nonzero: bool):
    return build(qkv_bias_nonzero, proj_bias_nonzero)


def prepare_inputs(x, qkv_w, q_bias, v_bias, rpb_table, proj_w, proj_b, rel_index):
    """Host-side prep: shard + transpose + fold scale + gather bias table."""
    x = np.asarray(x, dtype=np.float32)
    qkv_w = np.asarray(qkv_w, dtype=np.float32)
    q_bias = np.asarray(q_bias, dtype=np.float32)
    v_bias = np.asarray(v_bias, dtype=np.float32)
    rpb_table = np.asarray(rpb_table, dtype=np.float32)
    proj_w = np.asarray(proj_w, dtype=np.float32)
    proj_b = np.asarray(proj_b, dtype=np.float32)
    rel_index = np.asarray(rel_index)

    qw = qkv_w[0:DIM] * np.float32(SCALE)   # exact: SCALE is a power of two
    qkw_fm = np.concatenate([qw, qkv_w[DIM:2 * DIM]], axis=0).T  # [768, 1536]
    # block to [c][feature-part p][kc, m] so DMA rows are 1536B contiguous
    qkw_h = np.ascontiguousarray(
        qkw_fm.reshape(KC, 128, 2 * KC, 128).transpose(2, 1, 0, 3)
        .reshape(2 * KC, 128, DIM)).astype(ml_dtypes.bfloat16)
    vw_h = np.ascontiguousarray(qkv_w[2 * DIM:3 * DIM].T).astype(
        ml_dtypes.bfloat16)                                      # [768, 768]
    pw_h = np.ascontiguousarray(proj_w.T).astype(ml_dtypes.bfloat16)

    # bias[i, j, h] -> biasT[h, j, i]
    bias = rpb_table[rel_index].astype(np.float32)               # (197,197,12)
    biasT = bias.transpose(2, 1, 0)                              # (12, j, i)
    ebT = np.exp(biasT)
    # expB for j1 (j=0:128), both heads of each pair adjacent
    # per-head quadrant table: [h][j-part][jchunk][i]; j2 rows 69:128 -> 1.0
    ebq_h = np.ones((H, 128, 2, NT), dtype=np.float32)
    for h in range(H):
        ebq_h[h, :, 0, :] = ebT[h, 0:128, :]
        ebq_h[h, 0:NT2, 1, :] = ebT[h, 128:NT, :]
    ebq_h = ebq_h.reshape(H, 128, 2 * NT).astype(ml_dtypes.bfloat16)
    ebb_h = np.zeros((HP, 128, 453), dtype=np.float32)
    for hp in range(HP):
        ebb_h[hp, 0:128, 0:NT] = biasT[2 * hp, 0:128, :]
        ebb_h[hp, 0:NT2, 256:256 + NT] = biasT[2 * hp, 128:NT, :]
    ebb_h = ebb_h.astype(ml_dtypes.bfloat16)
    id_h = np.eye(128, dtype=ml_dtypes.bfloat16)

    qkv_bias_nonzero = bool(q_bias.any() or v_bias.any())
    proj_bias_nonzero = bool(proj_b.any())

    in_maps = []
    for i in range(NCORES):
        xs = x[i * BPC:(i + 1) * BPC].reshape(TOK, DIM)
        m = {
            "xt": np.ascontiguousarray(xs.T).astype(ml_dtypes.bfloat16),
            "qkw": qkw_h, "vw": vw_h, "pw": pw_h,
            "ebq": ebq_h, "ebb": ebb_h, "idm": id_h,
        }
        if qkv_bias_nonzero:
            m["qkb"] = np.ascontiguousarray(
                np.concatenate([q_bias * np.float32(SCALE),
                                np.zeros_like(q_bias)])[None, :],
                dtype=np.float32).astype(ml_dtypes.bfloat16)
            m["vb"] = np.ascontiguousarray(
                v_bias[None, :]).astype(ml_dtypes.bfloat16)
        if proj_bias_nonzero:
            m["pb"] = np.ascontiguousarray(
                proj_b[None, :], dtype=np.float32).astype(ml_dtypes.bfloat16)
        in_maps.append(m)
    return in_maps, qkv_bias_nonzero, proj_bias_nonzero


def kernel(x, qkv_w, q_bias, v_bias, rpb_table, proj_w, proj_b, rel_index):
    in_maps, qb_nz, pb_nz = prepare_inputs(
        x, qkv_w, q_bias, v_bias, rpb_table, proj_w, proj_b, rel_index)
    nc = _built(qb_nz, pb_nz)
    res = run_bass_kernel_spmd(nc, in_maps, core_ids=list(range(NCORES)))
    outs = []
    for i in range(NCORES):
        ofm = res.results[i]["out"].astype(np.float32)    # [768, 1576]
        outs.append(ofm.T.reshape(BPC, NT, DIM))
    return np.concatenate(outs, axis=0).astype(np.float32)
